# revision 5
# baseline (speedup 1.0000x reference)
"""Trainium2 Bass kernel for nn_Bottleneck_75325136437765 (sparse 3x3 local attention bottleneck).

Sharding: data-parallel over batch B=16 across 8 cores (2 batches/core), params replicated.

Per-core layout: channels on partitions, spatial (32*32=1024) on free dim.
  conv1/qkv/conv3: plain matmuls (lhsT = transposed weights, host-precomputed).
  attention:
    logits L[g,kk,hw] = sum_d q[gd,hw] * (k[gd,hw+off_kk] + pos[gd,kk])
      - products via DVE scalar_tensor_tensor: (k_window + pos) * q    (pos add fused)
      - d-group reduce (256ch -> 32 heads) via PE matmul with 0/1 selection lhsT
    softmax over kk (no max-subtraction; logits are small) with 1/sum factored out:
      out_pre[c,hw] = sum_kk exp(L)[g(c),kk,hw] * v[c,hw+off_kk]
      - exp(L) broadcast head->8 channels via PE expansion matmul (0/1 lhsT)
      - per-shift product on DVE; sum over kk via PE identity-matmul PSUM accumulation
      h2 = relu(out_pre * recip(sumexp)_bc + bnatt_b)
  residual + relu fused into conv3 epilogue on DVE.
BN is inference-mode scale/bias: scales folded into weights host-side, biases kept.
"""

import numpy as np

import concourse.bass as bass
import concourse.bacc as bacc
import concourse.tile as tile
from concourse import mybir
from concourse.bass_utils import run_bass_kernel_spmd

# ---- problem constants (hardcoded per contract) ----
B, CIN, H, W = 16, 1024, 32, 32
WIDTH, OUT, HEADS, KS = 256, 1024, 32, 3
D = WIDTH // HEADS            # 8 channels per head
HW = H * W                    # 1024
NC_ = 8                       # cores
BL = B // NC_                 # 2 batches per core
P = 128
KC1 = CIN // P                # 8 contraction chunks for conv1
PT = WIDTH // P               # 2 partition tiles for width-256 tensors
OC = OUT // P                 # 8 output ptiles for conv3
F32 = mybir.dt.float32
BF16 = mybir.dt.bfloat16
NHALF = 2                     # fp32 matmul free-dim limit 512 -> two halves of 1024


def _ns(n):
    return slice(n * 512, (n + 1) * 512)


def build_program():
    nc = bacc.Bacc(None, target_bir_lowering=False, debug=False)

    dr = {}
    def din(name, shape, dt=F32):
        dr[name] = nc.dram_tensor(name, list(shape), dt, kind="ExternalInput").ap()
        return dr[name]

    x_d = din("x", (BL, KC1, P, HW))
    w1T_d = din("w1T", (KC1, P, WIDTH))
    b1_d = din("b1", (PT, P, 1))
    wqT_d = din("wqT", (PT, P, WIDTH))
    wkT_d = din("wkT", (PT, P, WIDTH))
    wvT_d = din("wvT", (PT, P, WIDTH))
    bq_d = din("bq", (PT, P, 1))
    bk_d = din("bk", (PT, P, 1))
    bv_d = din("bv", (PT, P, 1))
    pos_d = din("pos", (PT, P, KS * KS))
    batt_d = din("batt", (PT, P, 1))
    w3T_d = din("w3T", (PT, P, OUT))
    b3_d = din("b3", (OC, P, 1))
    sel_d = din("sel", (PT, P, HEADS))
    expm_d = din("expm", (HEADS, PT, P), BF16)
    ident_d = din("ident", (P, P))
    out_d = nc.dram_tensor("out", [BL, OC, P, HW], F32, kind="ExternalOutput").ap()

    with tile.TileContext(nc) as tc:
        with (
            tc.tile_pool(name="consts", bufs=1) as consts,
            tc.tile_pool(name="xb", bufs=1) as xbp,
            tc.tile_pool(name="act", bufs=2) as actp,
            tc.tile_pool(name="pad", bufs=1) as padp,
            tc.tile_pool(name="attn", bufs=1) as attnp,
            tc.tile_pool(name="tmp", bufs=3) as tmpp,
            tc.tile_pool(name="tmp2", bufs=3) as tmp2p,
            tc.tile_pool(name="outz", bufs=2) as outzp,
            tc.tile_pool(name="pmm", bufs=2, space="PSUM") as pmm,
            tc.tile_pool(name="pL", bufs=1, space="PSUM") as pLp,
            tc.tile_pool(name="pacc", bufs=1, space="PSUM") as paccp,
        ):
            # ---- load constants ----
            w1T = consts.tile([P, KC1, WIDTH], F32)
            nc.sync.dma_start(out=w1T, in_=w1T_d.rearrange("k p m -> p k m"))
            wqT = consts.tile([P, PT, WIDTH], F32)
            nc.sync.dma_start(out=wqT, in_=wqT_d.rearrange("k p m -> p k m"))
            wkT = consts.tile([P, PT, WIDTH], F32)
            nc.sync.dma_start(out=wkT, in_=wkT_d.rearrange("k p m -> p k m"))
            wvT = consts.tile([P, PT, WIDTH], F32)
            nc.sync.dma_start(out=wvT, in_=wvT_d.rearrange("k p m -> p k m"))
            w3T = consts.tile([P, PT, OUT], F32)
            nc.sync.dma_start(out=w3T, in_=w3T_d.rearrange("k p m -> p k m"))
            b1 = consts.tile([P, PT, 1], F32)
            nc.sync.dma_start(out=b1, in_=b1_d.rearrange("t p o -> p t o"))
            bq = consts.tile([P, PT, 1], F32)
            nc.sync.dma_start(out=bq, in_=bq_d.rearrange("t p o -> p t o"))
            bk = consts.tile([P, PT, 1], F32)
            nc.sync.dma_start(out=bk, in_=bk_d.rearrange("t p o -> p t o"))
            bv = consts.tile([P, PT, 1], F32)
            nc.sync.dma_start(out=bv, in_=bv_d.rearrange("t p o -> p t o"))
            batt = consts.tile([P, PT, 1], F32)
            nc.sync.dma_start(out=batt, in_=batt_d.rearrange("t p o -> p t o"))
            b3 = consts.tile([P, OC, 1], F32)
            nc.sync.dma_start(out=b3, in_=b3_d.rearrange("t p o -> p t o"))
            pos = consts.tile([P, PT, KS * KS], F32)
            nc.sync.dma_start(out=pos, in_=pos_d.rearrange("t p o -> p t o"))
            sel = consts.tile([P, PT, HEADS], F32)
            nc.sync.dma_start(out=sel, in_=sel_d.rearrange("t p o -> p t o"))
            expm = consts.tile([HEADS, PT, P], BF16)
            nc.sync.dma_start(out=expm, in_=expm_d)
            ident = consts.tile([P, P], F32)
            nc.sync.dma_start(out=ident, in_=ident_d)

            for b in range(BL):
                # ---- load x ----
                xb = xbp.tile([P, KC1, HW], F32, tag="xb")
                nc.sync.dma_start(out=xb, in_=x_d[b].rearrange("k p n -> p k n"))

                # ---- conv1: h1 = relu(x @ w1' + b1) ----
                h1 = actp.tile([P, PT, HW], F32, tag="h1")
                for mc in range(PT):
                    ps = pmm.tile([P, HW], F32, tag="mm")
                    for n in range(NHALF):
                        for kc in range(KC1):
                            nc.tensor.matmul(
                                ps[:, _ns(n)],
                                w1T[:, kc, mc * P:(mc + 1) * P],
                                xb[:, kc, _ns(n)],
                                start=(kc == 0), stop=(kc == KC1 - 1),
                            )
                    nc.scalar.activation(
                        out=h1[:, mc, :], in_=ps,
                        func=mybir.ActivationFunctionType.Relu,
                        bias=b1[:, mc], scale=1.0,
                    )

                # ---- q/k/v convs ----
                q = actp.tile([P, PT, HW], F32, tag="q")
                kpad = padp.tile([P, PT, H + 2, W + 2], F32, tag="kpad")
                vpad = padp.tile([P, PT, H + 2, W + 2], F32, tag="vpad")
                nc.gpsimd.memset(kpad, 0.0)
                nc.gpsimd.memset(vpad, 0.0)
                for wT, bias, relu, dest in (
                    (wqT, bq, True, None),
                    (wkT, bk, True, kpad),
                    (wvT, bv, False, vpad),
                ):
                    for mc in range(PT):
                        ps = pmm.tile([P, HW], F32, tag="mm")
                        for n in range(NHALF):
                            for kc in range(PT):
                                nc.tensor.matmul(
                                    ps[:, _ns(n)],
                                    wT[:, kc, mc * P:(mc + 1) * P],
                                    h1[:, kc, _ns(n)],
                                    start=(kc == 0), stop=(kc == PT - 1),
                                )
                        if dest is None:
                            o, i = q[:, mc, :], ps[:]
                        else:
                            o = dest[:, mc, 1:H + 1, 1:W + 1]
                            i = ps.rearrange("p (a b) -> p a b", a=H)
                        nc.scalar.activation(
                            out=o, in_=i,
                            func=(mybir.ActivationFunctionType.Relu if relu
                                  else mybir.ActivationFunctionType.Identity),
                            bias=bias[:, mc], scale=1.0,
                        )

                # ---- attention logits + exp ----
                # e[g, kk, hw] = exp(L), bf16
                e = attnp.tile([HEADS, KS * KS, HW], BF16, tag="e")
                for kk in range(KS * KS):
                    di, dj = kk // KS, kk % KS
                    Lp = pLp.tile([HEADS, HW], F32, tag="L")
                    tmps = []
                    for pt in range(PT):
                        tmp = tmpp.tile([P, HW], F32, tag="tmp")
                        nc.vector.scalar_tensor_tensor(
                            out=tmp.rearrange("p (a b) -> p a b", a=H),
                            in0=kpad[:, pt, di:di + H, dj:dj + W],
                            scalar=pos[:, pt, kk:kk + 1],
                            in1=q[:, pt, :].rearrange("p (a b) -> p a b", a=H),
                            op0=mybir.AluOpType.add,
                            op1=mybir.AluOpType.mult,
                        )
                        tmps.append(tmp)
                    for n in range(NHALF):
                        for pt in range(PT):
                            nc.tensor.matmul(
                                Lp[:, _ns(n)],
                                sel[:, pt, :],
                                tmps[pt][:, _ns(n)],
                                start=(pt == 0), stop=(pt == PT - 1),
                            )
                    nc.scalar.activation(
                        out=e[:, kk, :], in_=Lp,
                        func=mybir.ActivationFunctionType.Exp,
                    )

                # ---- softmax denominator ----
                den = attnp.tile([HEADS, HW], F32, tag="den")
                nc.vector.tensor_reduce(
                    out=den, in_=e.rearrange("g k n -> g n k"),
                    axis=mybir.AxisListType.X, op=mybir.AluOpType.add,
                )
                recip = attnp.tile([HEADS, HW], F32, tag="recip")
                nc.vector.reciprocal(out=recip, in_=den)

                # recip broadcast head -> channels (PE expansion), copied to SBUF
                recip_bc = attnp.tile([P, PT, HW], F32, tag="recip_bc")
                rb16 = attnp.tile([HEADS, HW], BF16, tag="recip16")
                nc.vector.tensor_copy(rb16, recip)
                for mc in range(PT):
                    ps = pmm.tile([P, HW], F32, tag="mm")
                    for n in range(NHALF):
                        nc.tensor.matmul(ps[:, _ns(n)], expm[:, mc, :],
                                         rb16[:, _ns(n)], start=True, stop=True)
                    nc.vector.tensor_copy(recip_bc[:, mc, :], ps)

                # ---- v side: out_pre[c] = sum_kk e_bc * v_shift ----
                h2 = actp.tile([P, PT, HW], F32, tag="h2")
                for mc in range(PT):
                    acc = paccp.tile([P, HW], F32, tag="acc")
                    for kk in range(KS * KS):
                        di, dj = kk // KS, kk % KS
                        eb = pmm.tile([P, HW], F32, tag="mm")
                        for n in range(NHALF):
                            nc.tensor.matmul(
                                eb[:, _ns(n)], expm[:, mc, :], e[:, kk, _ns(n)],
                                start=True, stop=True,
                            )
                        t2 = tmp2p.tile([P, HW], F32, tag="tmp2")
                        nc.vector.tensor_tensor(
                            out=t2.rearrange("p (a b) -> p a b", a=H),
                            in0=eb.rearrange("p (a b) -> p a b", a=H),
                            in1=vpad[:, mc, di:di + H, dj:dj + W],
                            op=mybir.AluOpType.mult,
                        )
                        for n in range(NHALF):
                            nc.tensor.matmul(
                                acc[:, _ns(n)], ident, t2[:, _ns(n)],
                                start=(kk == 0), stop=(kk == KS * KS - 1),
                                skip_group_check=True,
                            )
                    # h2 = relu(acc * recip_bc + batt)
                    t3 = tmp2p.tile([P, HW], F32, tag="tmp2")
                    nc.vector.tensor_tensor(
                        out=t3, in0=acc, in1=recip_bc[:, mc, :],
                        op=mybir.AluOpType.mult,
                    )
                    nc.scalar.activation(
                        out=h2[:, mc, :], in_=t3,
                        func=mybir.ActivationFunctionType.Relu,
                        bias=batt[:, mc], scale=1.0,
                    )

                # ---- conv3 + residual + relu ----
                for oc in range(OC):
                    ps = pmm.tile([P, HW], F32, tag="mm")
                    for n in range(NHALF):
                        for kc in range(PT):
                            nc.tensor.matmul(
                                ps[:, _ns(n)],
                                w3T[:, kc, oc * P:(oc + 1) * P],
                                h2[:, kc, _ns(n)],
                                start=(kc == 0), stop=(kc == PT - 1),
                            )
                    z = outzp.tile([P, HW], F32, tag="outz")
                    # z = (x + b3) + psum
                    nc.vector.scalar_tensor_tensor(
                        out=z, in0=xb[:, oc, :], scalar=b3[:, oc], in1=ps,
                        op0=mybir.AluOpType.add, op1=mybir.AluOpType.add,
                    )
                    zr = outzp.tile([P, HW], F32, tag="outzr")
                    nc.gpsimd.tensor_scalar_max(zr, z, 0.0)
                    nc.sync.dma_start(out=out_d[b, oc], in_=zr)

    nc.compile()
    return nc


_PROG = None


def _host_prep(inputs):
    f = lambda a: np.asarray(a, dtype=np.float32)
    x = f(inputs["x"])
    # fold bn scales into weights (bn(conv(x,W),s,b) = conv(x, s*W) + b)
    w1 = f(inputs["w_conv1"]) * f(inputs["bn1_s"])[:, None]
    wq = f(inputs["wq"]) * f(inputs["bnq_s"])[:, None]
    wk = f(inputs["wk"]) * f(inputs["bnk_s"])[:, None]
    # fold bnatt scale through the (linear) attention-value path into v
    sv = f(inputs["bnatt_s"]) * f(inputs["bnv_s"])
    wv = f(inputs["wv"]) * sv[:, None]
    bv = f(inputs["bnatt_s"]) * f(inputs["bnv_b"])
    w3 = f(inputs["w_conv3"]) * f(inputs["bn3_s"])[:, None]

    posf = (f(inputs["pos_h"]) + f(inputs["pos_w"])).reshape(WIDTH, KS * KS)

    sel = np.zeros((PT, P, HEADS), np.float32)
    expm = np.zeros((HEADS, PT, P), np.float32)
    for pt in range(PT):
        for c in range(P):
            g = pt * (P // D) + c // D
            sel[pt, c, g] = 1.0
            expm[g, pt, c] = 1.0
    import ml_dtypes
    expm = expm.astype(ml_dtypes.bfloat16)

    com = {
        "w1T": np.ascontiguousarray(w1.T.reshape(KC1, P, WIDTH)),
        "b1": f(inputs["bn1_b"]).reshape(PT, P, 1),
        "wqT": np.ascontiguousarray(wq.T.reshape(PT, P, WIDTH)),
        "wkT": np.ascontiguousarray(wk.T.reshape(PT, P, WIDTH)),
        "wvT": np.ascontiguousarray(wv.T.reshape(PT, P, WIDTH)),
        "bq": f(inputs["bnq_b"]).reshape(PT, P, 1),
        "bk": f(inputs["bnk_b"]).reshape(PT, P, 1),
        "bv": bv.reshape(PT, P, 1),
        "pos": posf.reshape(PT, P, KS * KS),
        "batt": f(inputs["bnatt_b"]).reshape(PT, P, 1),
        "w3T": np.ascontiguousarray(w3.T.reshape(PT, P, OUT)),
        "b3": f(inputs["bn3_b"]).reshape(OC, P, 1),
        "sel": sel,
        "expm": expm,
        "ident": np.eye(P, dtype=np.float32),
    }
    xr = x.reshape(B, KC1, P, HW)
    in_maps = [dict(com, x=np.ascontiguousarray(xr[c * BL:(c + 1) * BL]))
               for c in range(NC_)]
    return in_maps


def kernel(**inputs):
    global _PROG
    if _PROG is None:
        _PROG = build_program()
    in_maps = _host_prep(inputs)
    res = run_bass_kernel_spmd(_PROG, in_maps, core_ids=list(range(NC_)))
    outs = [res.results[c]["out"].reshape(BL, OUT, H, W) for c in range(NC_)]
    return np.concatenate(outs, axis=0)


# revision 14
# speedup vs baseline: 1.7091x; 1.7091x over previous
"""Trainium2 Bass kernel for nn_Bottleneck_75325136437765 (sparse 3x3 local attention bottleneck).

Sharding: data-parallel over batch B=16 across 8 cores (2 batches/core), params replicated.

Per-core layout: channels on partitions, spatial (32*32=1024) on free dim.
  conv1/qkv/conv3: plain matmuls (lhsT = transposed weights, host-precomputed).
  attention:
    logits L[g,kk,hw] = sum_d q[gd,hw] * (k[gd,hw+off_kk] + pos[gd,kk])
      - products via DVE scalar_tensor_tensor: (k_window + pos) * q    (pos add fused)
      - d-group reduce (256ch -> 32 heads) via PE matmul with 0/1 selection lhsT
    softmax over kk (no max-subtraction; logits are small) with 1/sum factored out:
      out_pre[c,hw] = sum_kk exp(L)[g(c),kk,hw] * v[c,hw+off_kk]
      - exp(L) broadcast head->8 channels via PE expansion matmul (0/1 lhsT)
      - per-shift product on DVE; sum over kk via PE identity-matmul PSUM accumulation
      h2 = relu(out_pre * recip(sumexp)_bc + bnatt_b)
  residual + relu fused into conv3 epilogue on DVE.
BN is inference-mode scale/bias: scales folded into weights host-side, biases kept.
"""

import numpy as np

import concourse.bass as bass
import concourse.bacc as bacc
import concourse.tile as tile
from concourse import mybir
from concourse.bass_utils import run_bass_kernel_spmd

# ---- problem constants (hardcoded per contract) ----
B, CIN, H, W = 16, 1024, 32, 32
WIDTH, OUT, HEADS, KS = 256, 1024, 32, 3
D = WIDTH // HEADS            # 8 channels per head
HW = H * W                    # 1024
NC_ = 8                       # cores
BL = B // NC_                 # 2 batches per core
P = 128
KC1 = CIN // P                # 8 contraction chunks for conv1
PT = WIDTH // P               # 2 partition tiles for width-256 tensors
OC = OUT // P                 # 8 output ptiles for conv3
F32 = mybir.dt.float32
BF16 = mybir.dt.bfloat16
PDT = BF16                    # dtype for attention products (q/kpad/vpad/tmp/tmp2/sel/ident)
NHALF = 2                     # PSUM-bank matmul free-dim limit 512 -> two halves of 1024


def _ns(n):
    return slice(n * 512, (n + 1) * 512)


def build_program():
    nc = bacc.Bacc(None, target_bir_lowering=False, debug=False)

    dr = {}
    def din(name, shape, dt=F32):
        dr[name] = nc.dram_tensor(name, list(shape), dt, kind="ExternalInput").ap()
        return dr[name]

    x_d = din("x", (BL, KC1, P, HW))
    w1T_d = din("w1T", (KC1, P, WIDTH))
    b1_d = din("b1", (PT, P, 1))
    wqT_d = din("wqT", (PT, P, WIDTH))
    wkT_d = din("wkT", (PT, P, WIDTH))
    wvT_d = din("wvT", (PT, P, WIDTH))
    bq_d = din("bq", (PT, P, 1))
    bk_d = din("bk", (PT, P, 1))
    bv_d = din("bv", (PT, P, 1))
    pos_d = din("pos", (PT, P, KS * KS))
    batt_d = din("batt", (PT, P, 1))
    w3T_d = din("w3T", (PT, P, OUT))
    b3_d = din("b3", (OC, P, 1))
    sel_d = din("sel", (PT, P, HEADS), PDT)
    expm_d = din("expm", (HEADS, PT, P), BF16)
    ident_d = din("ident", (P, P), PDT)
    out_d = nc.dram_tensor("out", [BL, OC, P, HW], F32, kind="ExternalOutput").ap()

    with tile.TileContext(nc) as tc:
        with (
            tc.tile_pool(name="consts", bufs=1) as consts,
            tc.tile_pool(name="xb", bufs=2) as xbp,
            tc.tile_pool(name="act", bufs=2) as actp,
            tc.tile_pool(name="attn", bufs=1) as attnp,
            tc.tile_pool(name="tmp", bufs=3) as tmpp,
            tc.tile_pool(name="tmp2", bufs=3) as tmp2p,
            tc.tile_pool(name="outz", bufs=2) as outzp,
            tc.tile_pool(name="pmm", bufs=2, space="PSUM") as pmm,
            tc.tile_pool(name="pL", bufs=1, space="PSUM") as pLp,
            tc.tile_pool(name="pacc", bufs=1, space="PSUM") as paccp,
        ):
            # ---- load constants ----
            w1T = consts.tile([P, KC1, WIDTH], F32)
            nc.sync.dma_start(out=w1T, in_=w1T_d.rearrange("k p m -> p k m"))
            wqT = consts.tile([P, PT, WIDTH], F32)
            nc.sync.dma_start(out=wqT, in_=wqT_d.rearrange("k p m -> p k m"))
            wkT = consts.tile([P, PT, WIDTH], F32)
            nc.sync.dma_start(out=wkT, in_=wkT_d.rearrange("k p m -> p k m"))
            wvT = consts.tile([P, PT, WIDTH], F32)
            nc.sync.dma_start(out=wvT, in_=wvT_d.rearrange("k p m -> p k m"))
            w3T = consts.tile([P, PT, OUT], F32)
            nc.sync.dma_start(out=w3T, in_=w3T_d.rearrange("k p m -> p k m"))
            b1 = consts.tile([P, PT, 1], F32)
            nc.sync.dma_start(out=b1, in_=b1_d.rearrange("t p o -> p t o"))
            bq = consts.tile([P, PT, 1], F32)
            nc.sync.dma_start(out=bq, in_=bq_d.rearrange("t p o -> p t o"))
            bk = consts.tile([P, PT, 1], F32)
            nc.sync.dma_start(out=bk, in_=bk_d.rearrange("t p o -> p t o"))
            bv = consts.tile([P, PT, 1], F32)
            nc.sync.dma_start(out=bv, in_=bv_d.rearrange("t p o -> p t o"))
            batt = consts.tile([P, PT, 1], F32)
            nc.sync.dma_start(out=batt, in_=batt_d.rearrange("t p o -> p t o"))
            b3 = consts.tile([P, OC, 1], F32)
            nc.sync.dma_start(out=b3, in_=b3_d.rearrange("t p o -> p t o"))
            pos = consts.tile([P, PT, KS * KS], F32)
            nc.sync.dma_start(out=pos, in_=pos_d.rearrange("t p o -> p t o"))
            sel = consts.tile([P, PT, HEADS], PDT)
            nc.sync.dma_start(out=sel, in_=sel_d.rearrange("t p o -> p t o"))
            expm = consts.tile([HEADS, PT, P], BF16)
            nc.sync.dma_start(out=expm, in_=expm_d)
            ident = consts.tile([P, P], PDT)
            nc.sync.dma_start(out=ident, in_=ident_d)

            # persistent zero-padded k/v tiles (borders stay zero across batches)
            kpad = consts.tile([P, PT, H + 2, W + 2], PDT)
            vpad = consts.tile([P, PT, H + 2, W + 2], PDT)
            nc.vector.memset(kpad, 0.0)
            nc.vector.memset(vpad, 0.0)

            for b in range(BL):
                # ---- load x ----
                xb = xbp.tile([P, KC1, HW], F32, tag="xb")
                nc.sync.dma_start(out=xb, in_=x_d[b].rearrange("k p n -> p k n"))

                # ---- conv1: h1 = relu(x @ w1' + b1) ----
                h1 = actp.tile([P, PT, HW], F32, tag="h1")
                for mc in range(PT):
                    ps = pmm.tile([P, HW], F32, tag="mm")
                    for n in range(NHALF):
                        for kc in range(KC1):
                            nc.tensor.matmul(
                                ps[:, _ns(n)],
                                w1T[:, kc, mc * P:(mc + 1) * P],
                                xb[:, kc, _ns(n)],
                                start=(kc == 0), stop=(kc == KC1 - 1),
                            )
                    nc.scalar.activation(
                        out=h1[:, mc, :], in_=ps,
                        func=mybir.ActivationFunctionType.Relu,
                        bias=b1[:, mc], scale=1.0,
                    )

                # ---- q/k/v convs ----
                q = actp.tile([P, PT, HW], PDT, tag="q")
                for wT, bias, relu, dest in (
                    (wqT, bq, True, None),
                    (wkT, bk, True, kpad),
                    (wvT, bv, False, vpad),
                ):
                    for mc in range(PT):
                        ps = pmm.tile([P, HW], F32, tag="mm")
                        for n in range(NHALF):
                            for kc in range(PT):
                                nc.tensor.matmul(
                                    ps[:, _ns(n)],
                                    wT[:, kc, mc * P:(mc + 1) * P],
                                    h1[:, kc, _ns(n)],
                                    start=(kc == 0), stop=(kc == PT - 1),
                                )
                        if dest is None:
                            o, i = q[:, mc, :], ps[:]
                        else:
                            o = dest[:, mc, 1:H + 1, 1:W + 1]
                            i = ps.rearrange("p (a b) -> p a b", a=H)
                        nc.scalar.activation(
                            out=o, in_=i,
                            func=(mybir.ActivationFunctionType.Relu if relu
                                  else mybir.ActivationFunctionType.Identity),
                            bias=bias[:, mc], scale=1.0,
                        )

                # ---- attention logits + exp ----
                # e[g, kk, hw] = exp(L), bf16
                e = attnp.tile([HEADS, KS * KS, HW], BF16, tag="e")
                for kk in range(KS * KS):
                    di, dj = kk // KS, kk % KS
                    Lp = pLp.tile([HEADS, HW], F32, tag="L")
                    tmps = []
                    for pt in range(PT):
                        tmp = tmpp.tile([P, HW], PDT, tag="tmp")
                        nc.vector.scalar_tensor_tensor(
                            out=tmp.rearrange("p (a b) -> p a b", a=H),
                            in0=kpad[:, pt, di:di + H, dj:dj + W],
                            scalar=pos[:, pt, kk:kk + 1],
                            in1=q[:, pt, :].rearrange("p (a b) -> p a b", a=H),
                            op0=mybir.AluOpType.add,
                            op1=mybir.AluOpType.mult,
                        )
                        tmps.append(tmp)
                    for n in range(NHALF):
                        for pt in range(PT):
                            nc.tensor.matmul(
                                Lp[:, _ns(n)],
                                sel[:, pt, :],
                                tmps[pt][:, _ns(n)],
                                start=(pt == 0), stop=(pt == PT - 1),
                            )
                    nc.scalar.activation(
                        out=e[:, kk, :], in_=Lp,
                        func=mybir.ActivationFunctionType.Exp,
                    )

                # ---- softmax denominator ----
                den = attnp.tile([HEADS, HW], F32, tag="den")
                nc.vector.tensor_reduce(
                    out=den, in_=e.rearrange("g k n -> g n k"),
                    axis=mybir.AxisListType.X, op=mybir.AluOpType.add,
                )
                recip = attnp.tile([HEADS, HW], F32, tag="recip")
                nc.vector.reciprocal(out=recip, in_=den)

                # recip broadcast head -> channels (PE expansion), copied to SBUF
                recip_bc = attnp.tile([P, PT, HW], F32, tag="recip_bc")
                rb16 = attnp.tile([HEADS, HW], BF16, tag="recip16")
                nc.vector.tensor_copy(rb16, recip)
                for mc in range(PT):
                    ps = pmm.tile([P, HW], F32, tag="mm")
                    for n in range(NHALF):
                        nc.tensor.matmul(ps[:, _ns(n)], expm[:, mc, :],
                                         rb16[:, _ns(n)], start=True, stop=True)
                    nc.vector.tensor_copy(recip_bc[:, mc, :], ps)

                # ---- v side: out_pre[c] = sum_kk e_bc * v_shift ----
                h2 = actp.tile([P, PT, HW], F32, tag="h2")
                for mc in range(PT):
                    acc = paccp.tile([P, HW], F32, tag="acc")
                    for kk in range(KS * KS):
                        di, dj = kk // KS, kk % KS
                        eb = pmm.tile([P, HW], F32, tag="mm")
                        for n in range(NHALF):
                            nc.tensor.matmul(
                                eb[:, _ns(n)], expm[:, mc, :], e[:, kk, _ns(n)],
                                start=True, stop=True,
                            )
                        t2 = tmp2p.tile([P, HW], PDT, tag="tmp2")
                        nc.vector.tensor_tensor(
                            out=t2.rearrange("p (a b) -> p a b", a=H),
                            in0=eb.rearrange("p (a b) -> p a b", a=H),
                            in1=vpad[:, mc, di:di + H, dj:dj + W],
                            op=mybir.AluOpType.mult,
                        )
                        for n in range(NHALF):
                            nc.tensor.matmul(
                                acc[:, _ns(n)], ident, t2[:, _ns(n)],
                                start=(kk == 0), stop=(kk == KS * KS - 1),
                                skip_group_check=True,
                            )
                    # h2 = relu(acc * recip_bc + batt)
                    t3 = tmp2p.tile([P, HW], F32, tag="tmp2")
                    nc.vector.tensor_tensor(
                        out=t3, in0=acc, in1=recip_bc[:, mc, :],
                        op=mybir.AluOpType.mult,
                    )
                    nc.scalar.activation(
                        out=h2[:, mc, :], in_=t3,
                        func=mybir.ActivationFunctionType.Relu,
                        bias=batt[:, mc], scale=1.0,
                    )

                # ---- conv3 + residual + relu ----
                for oc in range(OC):
                    ps = pmm.tile([P, HW], F32, tag="mm")
                    for n in range(NHALF):
                        for kc in range(PT):
                            nc.tensor.matmul(
                                ps[:, _ns(n)],
                                w3T[:, kc, oc * P:(oc + 1) * P],
                                h2[:, kc, _ns(n)],
                                start=(kc == 0), stop=(kc == PT - 1),
                            )
                    z = outzp.tile([P, HW], F32, tag="outz")
                    # z = (x + b3) + psum
                    nc.vector.scalar_tensor_tensor(
                        out=z, in0=xb[:, oc, :], scalar=b3[:, oc], in1=ps,
                        op0=mybir.AluOpType.add, op1=mybir.AluOpType.add,
                    )
                    zr = outzp.tile([P, HW], F32, tag="outzr")
                    nc.scalar.activation(
                        out=zr, in_=z, func=mybir.ActivationFunctionType.Relu,
                    )
                    nc.sync.dma_start(out=out_d[b, oc], in_=zr)

    nc.compile()
    return nc


_PROG = None


def _host_prep(inputs):
    f = lambda a: np.asarray(a, dtype=np.float32)
    x = f(inputs["x"])
    # fold bn scales into weights (bn(conv(x,W),s,b) = conv(x, s*W) + b)
    w1 = f(inputs["w_conv1"]) * f(inputs["bn1_s"])[:, None]
    wq = f(inputs["wq"]) * f(inputs["bnq_s"])[:, None]
    wk = f(inputs["wk"]) * f(inputs["bnk_s"])[:, None]
    # fold bnatt scale through the (linear) attention-value path into v
    sv = f(inputs["bnatt_s"]) * f(inputs["bnv_s"])
    wv = f(inputs["wv"]) * sv[:, None]
    bv = f(inputs["bnatt_s"]) * f(inputs["bnv_b"])
    w3 = f(inputs["w_conv3"]) * f(inputs["bn3_s"])[:, None]

    posf = (f(inputs["pos_h"]) + f(inputs["pos_w"])).reshape(WIDTH, KS * KS)

    sel = np.zeros((PT, P, HEADS), np.float32)
    expm = np.zeros((HEADS, PT, P), np.float32)
    for pt in range(PT):
        for c in range(P):
            g = pt * (P // D) + c // D
            sel[pt, c, g] = 1.0
            expm[g, pt, c] = 1.0
    import ml_dtypes
    expm = expm.astype(ml_dtypes.bfloat16)

    com = {
        "w1T": np.ascontiguousarray(w1.T.reshape(KC1, P, WIDTH)),
        "b1": f(inputs["bn1_b"]).reshape(PT, P, 1),
        "wqT": np.ascontiguousarray(wq.T.reshape(PT, P, WIDTH)),
        "wkT": np.ascontiguousarray(wk.T.reshape(PT, P, WIDTH)),
        "wvT": np.ascontiguousarray(wv.T.reshape(PT, P, WIDTH)),
        "bq": f(inputs["bnq_b"]).reshape(PT, P, 1),
        "bk": f(inputs["bnk_b"]).reshape(PT, P, 1),
        "bv": bv.reshape(PT, P, 1),
        "pos": posf.reshape(PT, P, KS * KS),
        "batt": f(inputs["bnatt_b"]).reshape(PT, P, 1),
        "w3T": np.ascontiguousarray(w3.T.reshape(PT, P, OUT)),
        "b3": f(inputs["bn3_b"]).reshape(OC, P, 1),
        "sel": sel.astype(ml_dtypes.bfloat16),
        "expm": expm,
        "ident": np.eye(P, dtype=np.float32).astype(ml_dtypes.bfloat16),
    }
    xr = x.reshape(B, KC1, P, HW)
    in_maps = [dict(com, x=np.ascontiguousarray(xr[c * BL:(c + 1) * BL]))
               for c in range(NC_)]
    return in_maps


def kernel(**inputs):
    global _PROG
    if _PROG is None:
        _PROG = build_program()
    in_maps = _host_prep(inputs)
    res = run_bass_kernel_spmd(_PROG, in_maps, core_ids=list(range(NC_)))
    outs = [res.results[c]["out"].reshape(BL, OUT, H, W) for c in range(NC_)]
    return np.concatenate(outs, axis=0)


# revision 18
# speedup vs baseline: 2.7855x; 1.6298x over previous
"""Trainium2 Bass kernel for nn_Bottleneck_75325136437765 (sparse 3x3 local attention bottleneck).

Sharding: data-parallel over batch B=16 across 8 cores (2 batches/core), params replicated.

Per-core layout: channels on partitions, spatial (32*32=1024) on free dim. All matmuls bf16
(fp32 matmuls cost 2 PE passes on TRN2), fp32 PSUM accumulation everywhere.

  conv1/qkv/conv3: plain matmuls (lhsT = transposed weights, host-precomputed, bn scales folded).
  attention logits, packed PSUM layout (row = 32*(kk%4) + head, 3 tiles of 4 shifts):
      L[g,kk,hw] = sum_d q[gd,hw]*k[gd,hw+off_kk]  (col-tiled 0/1-selection matmuls over products)
                 + sum_d q[gd,hw]*pos[gd,kk]       (P2 matmul, accumulated into same PSUM)
  softmax over kk without max-subtraction, 1/sum factored out to the end:
      e = exp(L) (packed, 3 ACT ops); den = sum_kk e via 0/1 matmuls; recip = 1/den
      out_pre[c,hw] = sum_kk e_bc[c,kk,hw] * v[c,hw+off_kk]
        e_bc via row-tiled expansion matmuls; per-shift product on DVE;
        sum over kk via identity-matmul PSUM accumulation
      h2 = relu(out_pre * recip_bc + bnatt_b)
  residual: x streamed again in fp32, fused (x+b3)+psum on DVE, relu on ACT.
"""

import numpy as np

import concourse.bass as bass
import concourse.bacc as bacc
import concourse.tile as tile
from concourse import mybir
from concourse.bass_utils import run_bass_kernel_spmd

# ---- problem constants (hardcoded per contract) ----
B, CIN, H, W = 16, 1024, 32, 32
WIDTH, OUT, HEADS, KS = 256, 1024, 32, 3
D = WIDTH // HEADS            # 8 channels per head
HW = H * W                    # 1024
NC_ = 8                       # cores
BL = B // NC_                 # 2 batches per core
P = 128
KC1 = CIN // P                # 8 contraction chunks for conv1
PT = WIDTH // P               # 2 partition tiles for width-256 tensors
OC = OUT // P                 # 8 output ptiles for conv3
NKK = KS * KS                 # 9 shifts
NT = 3                        # packed logit tiles (4+4+1 shifts)
F32 = mybir.dt.float32
BF16 = mybir.dt.bfloat16
NHALF = 2                     # PSUM-bank limit: matmul N<=512 fp32 out


def _ns(n):
    return slice(n * 512, (n + 1) * 512)


def build_program():
    nc = bacc.Bacc(None, target_bir_lowering=False, debug=False)

    def din(name, shape, dt=BF16):
        return nc.dram_tensor(name, list(shape), dt, kind="ExternalInput").ap()

    x16_d = din("x16", (BL, KC1, P, HW))
    xf_d = din("xf", (BL, OC, P, HW), F32)
    w1T_d = din("w1T", (KC1, P, WIDTH))
    wqT_d = din("wqT", (PT, P, WIDTH))
    wkT_d = din("wkT", (PT, P, WIDTH))
    wvT_d = din("wvT", (PT, P, WIDTH))
    w3T_d = din("w3T", (PT, P, OUT))
    b1_d = din("b1", (PT, P, 1), F32)
    bq_d = din("bq", (PT, P, 1), F32)
    bk_d = din("bk", (PT, P, 1), F32)
    bv_d = din("bv", (PT, P, 1), F32)
    batt_d = din("batt", (PT, P, 1), F32)
    b3_d = din("b3", (OC, P, 1), F32)
    sel_d = din("sel", (PT, P, HEADS))
    p2_d = din("p2", (PT, P, NT, P))
    sab_d = din("sab", (P, HEADS))
    eye32_d = din("eye32", (HEADS, HEADS))
    expm4_d = din("expm4", (P, PT, P))
    ident_d = din("ident", (P, P))
    out_d = nc.dram_tensor("out", [BL, OC, P, HW], F32, kind="ExternalOutput").ap()

    with tile.TileContext(nc) as tc:
        with (
            tc.tile_pool(name="consts", bufs=1) as consts,
            tc.tile_pool(name="xb", bufs=2) as xbp,
            tc.tile_pool(name="xres", bufs=3) as xresp,
            tc.tile_pool(name="act", bufs=2) as actp,
            tc.tile_pool(name="attn", bufs=1) as attnp,
            tc.tile_pool(name="epk", bufs=4) as epkp,
            tc.tile_pool(name="tmp", bufs=10) as tmpp,
            tc.tile_pool(name="tmp2", bufs=4) as tmp2p,
            tc.tile_pool(name="outz", bufs=3) as outzp,
            tc.tile_pool(name="pmm", bufs=2, space="PSUM") as pmm,
            tc.tile_pool(name="pL", bufs=1, space="PSUM") as pLp,
            tc.tile_pool(name="pacc", bufs=1, space="PSUM") as paccp,
        ):
            # ---- load constants ----
            def cload(name, dram, shape, dt=BF16, re="k p m -> p k m"):
                t = consts.tile(shape, dt, tag=name)
                nc.sync.dma_start(out=t, in_=dram.rearrange(re) if re else dram)
                return t

            w1T = cload("w1T", w1T_d, [P, KC1, WIDTH])
            wqT = cload("wqT", wqT_d, [P, PT, WIDTH])
            wkT = cload("wkT", wkT_d, [P, PT, WIDTH])
            wvT = cload("wvT", wvT_d, [P, PT, WIDTH])
            w3T = cload("w3T", w3T_d, [P, PT, OUT])
            b1 = cload("b1", b1_d, [P, PT, 1], F32)
            bq = cload("bq", bq_d, [P, PT, 1], F32)
            bk = cload("bk", bk_d, [P, PT, 1], F32)
            bv = cload("bv", bv_d, [P, PT, 1], F32)
            batt = cload("batt", batt_d, [P, PT, 1], F32)
            b3 = cload("b3", b3_d, [P, OC, 1], F32)
            sel = cload("sel", sel_d, [P, PT, HEADS])
            p2 = cload("p2", p2_d, [P, PT, NT, P], re="k p m o -> p k m o")
            sab = cload("sab", sab_d, [P, HEADS], re=None)
            eye32 = cload("eye32", eye32_d, [HEADS, HEADS], re=None)
            expm4 = cload("expm4", expm4_d, [P, PT, P], re=None)
            ident = cload("ident", ident_d, [P, P], re=None)

            # persistent zero-padded k/v tiles (borders stay zero across batches)
            kpad = consts.tile([P, PT, H + 2, W + 2], BF16)
            vpad = consts.tile([P, PT, H + 2, W + 2], BF16)
            nc.vector.memset(kpad, 0.0)
            nc.vector.memset(vpad, 0.0)

            for b in range(BL):
                # ---- load x (bf16 for conv1) ----
                xb = xbp.tile([P, KC1, HW], BF16, tag="xb")
                nc.sync.dma_start(out=xb, in_=x16_d[b].rearrange("k p n -> p k n"))

                # ---- conv1: h1 = relu(x @ w1' + b1) ----
                h1 = actp.tile([P, PT, HW], BF16, tag="h1")
                for mc in range(PT):
                    ps = pmm.tile([P, HW], F32, tag="mm")
                    for kc in range(KC1):
                        for n in range(NHALF):
                            nc.tensor.matmul(
                                ps[:, _ns(n)],
                                w1T[:, kc, mc * P:(mc + 1) * P],
                                xb[:, kc, _ns(n)],
                                start=(kc == 0), stop=(kc == KC1 - 1),
                            )
                    nc.scalar.activation(
                        out=h1[:, mc, :], in_=ps,
                        func=mybir.ActivationFunctionType.Relu,
                        bias=b1[:, mc], scale=1.0,
                    )

                # ---- q/k/v convs ----
                q = actp.tile([P, PT, HW], BF16, tag="q")
                for wT, bias, relu, dest in (
                    (wqT, bq, True, None),
                    (wkT, bk, True, kpad),
                    (wvT, bv, False, vpad),
                ):
                    for mc in range(PT):
                        ps = pmm.tile([P, HW], F32, tag="mm")
                        for kc in range(PT):
                            for n in range(NHALF):
                                nc.tensor.matmul(
                                    ps[:, _ns(n)],
                                    wT[:, kc, mc * P:(mc + 1) * P],
                                    h1[:, kc, _ns(n)],
                                    start=(kc == 0), stop=(kc == PT - 1),
                                )
                        if dest is None:
                            o, i = q[:, mc, :], ps[:]
                        else:
                            o = dest[:, mc, 1:H + 1, 1:W + 1]
                            i = ps.rearrange("p (a b) -> p a b", a=H)
                        nc.scalar.activation(
                            out=o, in_=i,
                            func=(mybir.ActivationFunctionType.Relu if relu
                                  else mybir.ActivationFunctionType.Identity),
                            bias=bias[:, mc], scale=1.0,
                        )

                # ---- attention logits (packed), exp, denominator ----
                # packed tile t rows: 32*(kk%4) + g  for kk in {4t..4t+3}
                epks = []
                den = attnp.tile([HEADS, HW], F32, tag="den")
                denp = paccp.tile([HEADS, HW], F32, tag="acc")
                for t in range(NT):
                    nsh = 4 if t < 2 else 1
                    rows = 32 * nsh
                    Lpk = pLp.tile([P, HW], F32, tag="Lpk")
                    # qpos term: all rows at once per pt chunk
                    for n in range(NHALF):
                        for pt in range(PT):
                            nc.tensor.matmul(
                                Lpk[:rows, _ns(n)],
                                p2[:, pt, t, :rows],
                                q[:, pt, _ns(n)],
                                start=(pt == 0), stop=False,
                                skip_group_check=True,
                            )
                    # qk products + col-tiled group reduce
                    for j in range(nsh):
                        kk = 4 * t + j
                        di, dj = kk // KS, kk % KS
                        for pt in range(PT):
                            tmp = tmpp.tile([P, HW], BF16, tag="tmp")
                            nc.vector.tensor_tensor(
                                out=tmp.rearrange("p (a b) -> p a b", a=H),
                                in0=kpad[:, pt, di:di + H, dj:dj + W],
                                in1=q[:, pt, :].rearrange("p (a b) -> p a b", a=H),
                                op=mybir.AluOpType.mult,
                            )
                            for n in range(NHALF):
                                nc.tensor.matmul(
                                    Lpk[32 * j:32 * (j + 1), _ns(n)],
                                    sel[:, pt, :],
                                    tmp[:, _ns(n)],
                                    start=False, stop=(pt == PT - 1),
                                    tile_position=(0, 32 * j),
                                    skip_group_check=True,
                                )
                    epk = epkp.tile([P, HW], BF16, tag="epk")
                    nc.scalar.activation(
                        out=epk[:rows, :], in_=Lpk[:rows, :],
                        func=mybir.ActivationFunctionType.Exp,
                    )
                    epks.append(epk)
                    # denominator accumulation
                    lhs = sab if t < 2 else eye32
                    for n in range(NHALF):
                        nc.tensor.matmul(
                            denp[:, _ns(n)], lhs[:rows, :], epk[:rows, _ns(n)],
                            start=(t == 0), stop=(t == NT - 1),
                            skip_group_check=True,
                        )
                nc.vector.reciprocal(out=den, in_=denp)
                rb16 = attnp.tile([HEADS, HW], BF16, tag="recip16")
                nc.vector.tensor_copy(rb16, den)

                # recip broadcast head -> channels (PE expansion), copied to SBUF
                recip_bc = attnp.tile([P, PT, HW], F32, tag="recip_bc")
                for mc in range(PT):
                    ps = pmm.tile([P, HW], F32, tag="mm")
                    for n in range(NHALF):
                        nc.tensor.matmul(ps[:, _ns(n)], expm4[:HEADS, mc, :],
                                         rb16[:, _ns(n)], start=True, stop=True)
                    nc.vector.tensor_copy(recip_bc[:, mc, :], ps)

                # ---- v side: out_pre[c] = sum_kk e_bc * v_shift ----
                h2 = actp.tile([P, PT, HW], BF16, tag="h2")
                for mc in range(PT):
                    acc = paccp.tile([P, HW], F32, tag="acc")
                    for kk in range(NKK):
                        t, j = kk // 4, kk % 4
                        di, dj = kk // KS, kk % KS
                        eb = pmm.tile([P, HW], F32, tag="mm")
                        for n in range(NHALF):
                            nc.tensor.matmul(
                                eb[:, _ns(n)],
                                expm4[32 * j:32 * (j + 1), mc, :],
                                epks[t][32 * j:32 * (j + 1), _ns(n)],
                                start=True, stop=True,
                                tile_position=(32 * j, 0),
                            )
                        t2 = tmp2p.tile([P, HW], BF16, tag="tmp2")
                        nc.vector.tensor_tensor(
                            out=t2.rearrange("p (a b) -> p a b", a=H),
                            in0=eb.rearrange("p (a b) -> p a b", a=H),
                            in1=vpad[:, mc, di:di + H, dj:dj + W],
                            op=mybir.AluOpType.mult,
                        )
                        for n in range(NHALF):
                            nc.tensor.matmul(
                                acc[:, _ns(n)], ident, t2[:, _ns(n)],
                                start=(kk == 0), stop=(kk == NKK - 1),
                                skip_group_check=True,
                            )
                    # h2 = relu(acc * recip_bc + batt)
                    t3 = tmp2p.tile([P, HW], F32, tag="t3")
                    nc.vector.tensor_tensor(
                        out=t3, in0=acc, in1=recip_bc[:, mc, :],
                        op=mybir.AluOpType.mult,
                    )
                    nc.scalar.activation(
                        out=h2[:, mc, :], in_=t3,
                        func=mybir.ActivationFunctionType.Relu,
                        bias=batt[:, mc], scale=1.0,
                    )

                # ---- conv3 + residual + relu ----
                for oc in range(OC):
                    xr = xresp.tile([P, HW], F32, tag="xres")
                    nc.sync.dma_start(out=xr, in_=xf_d[b, oc])
                    ps = pmm.tile([P, HW], F32, tag="mm")
                    for kc in range(PT):
                        for n in range(NHALF):
                            nc.tensor.matmul(
                                ps[:, _ns(n)],
                                w3T[:, kc, oc * P:(oc + 1) * P],
                                h2[:, kc, _ns(n)],
                                start=(kc == 0), stop=(kc == PT - 1),
                            )
                    z = outzp.tile([P, HW], F32, tag="outz")
                    # z = (x + b3) + psum
                    nc.vector.scalar_tensor_tensor(
                        out=z, in0=xr, scalar=b3[:, oc], in1=ps,
                        op0=mybir.AluOpType.add, op1=mybir.AluOpType.add,
                    )
                    zr = outzp.tile([P, HW], F32, tag="outzr")
                    nc.scalar.activation(
                        out=zr, in_=z, func=mybir.ActivationFunctionType.Relu,
                    )
                    nc.sync.dma_start(out=out_d[b, oc], in_=zr)

    nc.compile()
    return nc


_PROG = None


def _host_prep(inputs):
    import ml_dtypes
    bf = ml_dtypes.bfloat16
    f = lambda a: np.asarray(a, dtype=np.float32)
    x = f(inputs["x"])
    # fold bn scales into weights (bn(conv(x,W),s,b) = conv(x, s*W) + b)
    w1 = f(inputs["w_conv1"]) * f(inputs["bn1_s"])[:, None]
    wq = f(inputs["wq"]) * f(inputs["bnq_s"])[:, None]
    wk = f(inputs["wk"]) * f(inputs["bnk_s"])[:, None]
    # fold bnatt scale through the (linear) attention-value path into v
    sv = f(inputs["bnatt_s"]) * f(inputs["bnv_s"])
    wv = f(inputs["wv"]) * sv[:, None]
    bv = f(inputs["bnatt_s"]) * f(inputs["bnv_b"])
    w3 = f(inputs["w_conv3"]) * f(inputs["bn3_s"])[:, None]

    posf = (f(inputs["pos_h"]) + f(inputs["pos_w"])).reshape(WIDTH, NKK)

    sel = np.zeros((PT, P, HEADS), np.float32)
    for pt in range(P // 64):
        pass
    for pt in range(PT):
        for c in range(P):
            sel[pt, c, pt * (P // D) + c // D] = 1.0
    # p2[pt, c, t, 32*j+g] = pos[c_global, 4t+j] if head(c_global)==g
    p2 = np.zeros((PT, P, NT, P), np.float32)
    for pt in range(PT):
        for c in range(P):
            g = pt * (P // D) + c // D
            for kk in range(NKK):
                t, j = kk // 4, kk % 4
                p2[pt, c, t, 32 * j + g] = posf[pt * P + c, kk]
    # sab[r, g] = 1 if r % 32 == g (sum over the 4 packed kk rows)
    sab = np.zeros((P, HEADS), np.float32)
    for r in range(P):
        sab[r, r % HEADS] = 1.0
    # expm4[32j+g, mc, c] = 1 if head(mc*128+c) == g   (replicated per row strip)
    expm4 = np.zeros((P, PT, P), np.float32)
    for jj in range(4):
        for g in range(HEADS):
            for mc in range(PT):
                for c in range(P):
                    if mc * (P // D) + c // D == g:
                        expm4[32 * jj + g, mc, c] = 1.0

    com = {
        "w1T": np.ascontiguousarray(w1.T.reshape(KC1, P, WIDTH)).astype(bf),
        "wqT": np.ascontiguousarray(wq.T.reshape(PT, P, WIDTH)).astype(bf),
        "wkT": np.ascontiguousarray(wk.T.reshape(PT, P, WIDTH)).astype(bf),
        "wvT": np.ascontiguousarray(wv.T.reshape(PT, P, WIDTH)).astype(bf),
        "w3T": np.ascontiguousarray(w3.T.reshape(PT, P, OUT)).astype(bf),
        "b1": f(inputs["bn1_b"]).reshape(PT, P, 1),
        "bq": f(inputs["bnq_b"]).reshape(PT, P, 1),
        "bk": f(inputs["bnk_b"]).reshape(PT, P, 1),
        "bv": bv.reshape(PT, P, 1),
        "batt": f(inputs["bnatt_b"]).reshape(PT, P, 1),
        "b3": f(inputs["bn3_b"]).reshape(OC, P, 1),
        "sel": sel.astype(bf),
        "p2": p2.astype(bf),
        "sab": sab.astype(bf),
        "eye32": np.eye(HEADS, dtype=np.float32).astype(bf),
        "expm4": expm4.astype(bf),
        "ident": np.eye(P, dtype=np.float32).astype(bf),
    }
    xr = x.reshape(B, KC1, P, HW)
    in_maps = []
    for c in range(NC_):
        xs = np.ascontiguousarray(xr[c * BL:(c + 1) * BL])
        in_maps.append(dict(com, x16=xs.astype(bf), xf=xs))
    return in_maps


def kernel(**inputs):
    global _PROG
    if _PROG is None:
        _PROG = build_program()
    in_maps = _host_prep(inputs)
    res = run_bass_kernel_spmd(_PROG, in_maps, core_ids=list(range(NC_)))
    outs = [res.results[c]["out"].reshape(BL, OUT, H, W) for c in range(NC_)]
    return np.concatenate(outs, axis=0)


# revision 24
# speedup vs baseline: 2.9091x; 1.0444x over previous
"""Trainium2 Bass kernel for nn_Bottleneck_75325136437765 (sparse 3x3 local attention bottleneck).

Sharding: data-parallel over batch B=16 across 8 cores (2 batches/core), params replicated.

Per-core layout: channels on partitions, spatial (32*32=1024) on free dim. All matmuls bf16
(fp32 matmuls cost 2 PE passes on TRN2), fp32 PSUM accumulation everywhere.

  conv1/qkv/conv3: plain matmuls (lhsT = transposed weights, host-precomputed, bn scales folded).
  attention logits, packed PSUM layout (row = 32*(kk%4) + head, 3 tiles of 4 shifts):
      L[g,kk,hw] = sum_d q[gd,hw]*k[gd,hw+off_kk]  (col-tiled 0/1-selection matmuls over products)
                 + sum_d q[gd,hw]*pos[gd,kk]       (P2 matmul, accumulated into same PSUM)
  softmax over kk without max-subtraction, 1/sum factored out to the end:
      e = exp(L) (packed, 3 ACT ops); den = sum_kk e via 0/1 matmuls; recip = 1/den
      out_pre[c,hw] = sum_kk e_bc[c,kk,hw] * v[c,hw+off_kk]
        e_bc via row-tiled expansion matmuls; per-shift product on DVE;
        sum over kk via identity-matmul PSUM accumulation
      h2 = relu(out_pre * recip_bc + bnatt_b)
  residual: x streamed again in fp32, fused (x+b3)+psum on DVE, relu on ACT.
"""

import numpy as np

import concourse.bass as bass
import concourse.bacc as bacc
import concourse.tile as tile
from concourse import mybir
from concourse.bass_utils import run_bass_kernel_spmd

# ---- problem constants (hardcoded per contract) ----
B, CIN, H, W = 16, 1024, 32, 32
WIDTH, OUT, HEADS, KS = 256, 1024, 32, 3
D = WIDTH // HEADS            # 8 channels per head
HW = H * W                    # 1024
NC_ = 8                       # cores
BL = B // NC_                 # 2 batches per core
P = 128
KC1 = CIN // P                # 8 contraction chunks for conv1
PT = WIDTH // P               # 2 partition tiles for width-256 tensors
OC = OUT // P                 # 8 output ptiles for conv3
NKK = KS * KS                 # 9 shifts
NT = 3                        # packed logit tiles (4+4+1 shifts)
F32 = mybir.dt.float32
BF16 = mybir.dt.bfloat16
NHALF = 2                     # PSUM-bank limit: matmul N<=512 fp32 out


def _ns(n):
    return slice(n * 512, (n + 1) * 512)


def build_program():
    nc = bacc.Bacc(None, target_bir_lowering=False, debug=False)

    def din(name, shape, dt=BF16):
        return nc.dram_tensor(name, list(shape), dt, kind="ExternalInput").ap()

    x16_d = din("x16", (BL, KC1, P, HW))
    xf_d = din("xf", (BL, OC, P, HW), F32)
    w1T_d = din("w1T", (KC1, P, WIDTH))
    wqT_d = din("wqT", (PT, P, WIDTH))
    wkT_d = din("wkT", (PT, P, WIDTH))
    wvT_d = din("wvT", (PT, P, WIDTH))
    w3T_d = din("w3T", (PT, P, OUT))
    b1_d = din("b1", (PT, P, 1), F32)
    bq_d = din("bq", (PT, P, 1), F32)
    bk_d = din("bk", (PT, P, 1), F32)
    bv_d = din("bv", (PT, P, 1), F32)
    batt_d = din("batt", (PT, P, 1), F32)
    b3_d = din("b3", (OC, P, 1), F32)
    sel_d = din("sel", (PT, P, HEADS))
    p2_d = din("p2", (PT, P, NT, P))
    sab_d = din("sab", (P, HEADS))
    eye32_d = din("eye32", (HEADS, HEADS))
    expm4_d = din("expm4", (P, PT, P))
    ident_d = din("ident", (P, P))
    out_d = nc.dram_tensor("out", [BL, OC, P, HW], F32, kind="ExternalOutput").ap()

    with tile.TileContext(nc) as tc:
        with (
            tc.tile_pool(name="consts", bufs=1) as consts,
            tc.tile_pool(name="xb", bufs=2) as xbp,
            tc.tile_pool(name="xres", bufs=3) as xresp,
            tc.tile_pool(name="act", bufs=2) as actp,
            tc.tile_pool(name="attn", bufs=1) as attnp,
            tc.tile_pool(name="epk", bufs=4) as epkp,
            tc.tile_pool(name="tmp", bufs=10) as tmpp,
            tc.tile_pool(name="tmp2", bufs=4) as tmp2p,
            tc.tile_pool(name="outz", bufs=3) as outzp,
            tc.tile_pool(name="pmm", bufs=2, space="PSUM") as pmm,
            tc.tile_pool(name="pL", bufs=1, space="PSUM") as pLp,
            tc.tile_pool(name="pacc", bufs=1, space="PSUM") as paccp,
        ):
            # ---- load constants ----
            def cload(name, dram, shape, dt=BF16, re="k p m -> p k m"):
                t = consts.tile(shape, dt, tag=name)
                nc.sync.dma_start(out=t, in_=dram.rearrange(re) if re else dram)
                return t

            w1T = consts.tile([P, KC1, WIDTH], BF16, tag="w1T")
            for kc in range(KC1):
                nc.sync.dma_start(out=w1T[:, kc, :], in_=w1T_d[kc])
            wqT = cload("wqT", wqT_d, [P, PT, WIDTH])
            wkT = cload("wkT", wkT_d, [P, PT, WIDTH])
            wvT = cload("wvT", wvT_d, [P, PT, WIDTH])
            w3T = cload("w3T", w3T_d, [P, PT, OUT])
            b1 = cload("b1", b1_d, [P, PT, 1], F32)
            bq = cload("bq", bq_d, [P, PT, 1], F32)
            bk = cload("bk", bk_d, [P, PT, 1], F32)
            bv = cload("bv", bv_d, [P, PT, 1], F32)
            batt = cload("batt", batt_d, [P, PT, 1], F32)
            b3 = cload("b3", b3_d, [P, OC, 1], F32)
            sel = cload("sel", sel_d, [P, PT, HEADS])
            p2 = cload("p2", p2_d, [P, PT, NT, P], re="k p m o -> p k m o")
            sab = cload("sab", sab_d, [P, HEADS], re=None)
            eye32 = cload("eye32", eye32_d, [HEADS, HEADS], re=None)
            expm4 = cload("expm4", expm4_d, [P, PT, P], re=None)
            ident = cload("ident", ident_d, [P, P], re=None)

            # persistent zero-padded k/v tiles (borders stay zero across batches)
            # kpad1 is kpad shifted left one column so dj=1 windows are 4B-aligned
            kpad = consts.tile([P, PT, H + 2, W + 2], BF16)
            kpad1 = consts.tile([P, PT, H + 2, W + 2], BF16)
            vpad = consts.tile([P, PT, H + 2, W + 2], BF16)
            nc.vector.memset(kpad, 0.0)
            nc.vector.memset(kpad1, 0.0)
            nc.vector.memset(vpad, 0.0)

            for b in range(BL):
                # ---- load x (bf16 for conv1), chunked so conv1 starts early ----
                xb = xbp.tile([P, KC1, HW], BF16, tag="xb")
                for kc in range(KC1):
                    nc.sync.dma_start(out=xb[:, kc, :], in_=x16_d[b, kc])

                # ---- conv1: h1 = relu(x @ w1' + b1) ----
                h1 = actp.tile([P, PT, HW], BF16, tag="h1")
                for mc in range(PT):
                    ps = pmm.tile([P, HW], F32, tag="mm")
                    for kc in range(KC1):
                        for n in range(NHALF):
                            nc.tensor.matmul(
                                ps[:, _ns(n)],
                                w1T[:, kc, mc * P:(mc + 1) * P],
                                xb[:, kc, _ns(n)],
                                start=(kc == 0), stop=(kc == KC1 - 1),
                            )
                    nc.scalar.activation(
                        out=h1[:, mc, :], in_=ps,
                        func=mybir.ActivationFunctionType.Relu,
                        bias=b1[:, mc], scale=1.0,
                    )

                # ---- q/k/v convs ----
                q = actp.tile([P, PT, HW], BF16, tag="q")
                for wT, bias, relu, dest in (
                    (wqT, bq, True, None),
                    (wkT, bk, True, kpad),
                    (wvT, bv, False, vpad),
                ):
                    for mc in range(PT):
                        ps = pmm.tile([P, HW], F32, tag="mm")
                        for kc in range(PT):
                            for n in range(NHALF):
                                nc.tensor.matmul(
                                    ps[:, _ns(n)],
                                    wT[:, kc, mc * P:(mc + 1) * P],
                                    h1[:, kc, _ns(n)],
                                    start=(kc == 0), stop=(kc == PT - 1),
                                )
                        if dest is None:
                            o, i = q[:, mc, :], ps[:]
                        else:
                            o = dest[:, mc, 1:H + 1, 1:W + 1]
                            i = ps.rearrange("p (a b) -> p a b", a=H)
                        nc.scalar.activation(
                            out=o, in_=i,
                            func=(mybir.ActivationFunctionType.Relu if relu
                                  else mybir.ActivationFunctionType.Identity),
                            bias=bias[:, mc], scale=1.0,
                        )
                        if dest is kpad:
                            nc.scalar.activation(
                                out=kpad1[:, mc, 1:H + 1, 0:W],
                                in_=ps.rearrange("p (a b) -> p a b", a=H),
                                func=mybir.ActivationFunctionType.Relu,
                                bias=bias[:, mc], scale=1.0,
                            )

                # ---- attention logits (packed), exp, denominator ----
                # packed tile t rows: 32*(kk%4) + g  for kk in {4t..4t+3}
                epks = []
                den = attnp.tile([HEADS, HW], F32, tag="den")
                denp = paccp.tile([HEADS, HW], F32, tag="acc")
                for t in range(NT):
                    nsh = 4 if t < 2 else 1
                    rows = 32 * nsh
                    Lpk = pLp.tile([P, HW], F32, tag="Lpk")
                    # qpos term: all rows at once per pt chunk
                    for n in range(NHALF):
                        for pt in range(PT):
                            nc.tensor.matmul(
                                Lpk[:rows, _ns(n)],
                                p2[:, pt, t, :rows],
                                q[:, pt, _ns(n)],
                                start=(pt == 0), stop=False,
                                skip_group_check=True,
                            )
                    # qk products + col-tiled group reduce
                    for j in range(nsh):
                        kk = 4 * t + j
                        di, dj = kk // KS, kk % KS
                        # odd dj is 2-byte-misaligned in kpad; use the shifted copy
                        ksrc, djs = (kpad, dj) if dj % 2 == 0 else (kpad1, dj - 1)
                        for pt in range(PT):
                            tmp = tmpp.tile([P, HW], BF16, tag="tmp")
                            nc.vector.tensor_tensor(
                                out=tmp.rearrange("p (a b) -> p a b", a=H),
                                in0=ksrc[:, pt, di:di + H, djs:djs + W],
                                in1=q[:, pt, :].rearrange("p (a b) -> p a b", a=H),
                                op=mybir.AluOpType.mult,
                            )
                            for n in range(NHALF):
                                nc.tensor.matmul(
                                    Lpk[32 * j:32 * (j + 1), _ns(n)],
                                    sel[:, pt, :],
                                    tmp[:, _ns(n)],
                                    start=False, stop=(pt == PT - 1),
                                    tile_position=(0, 32 * j),
                                    skip_group_check=True,
                                )
                    epk = epkp.tile([P, HW], BF16, tag="epk")
                    nc.scalar.activation(
                        out=epk[:rows, :], in_=Lpk[:rows, :],
                        func=mybir.ActivationFunctionType.Exp,
                    )
                    epks.append(epk)
                    # denominator accumulation
                    lhs = sab if t < 2 else eye32
                    for n in range(NHALF):
                        nc.tensor.matmul(
                            denp[:, _ns(n)], lhs[:rows, :], epk[:rows, _ns(n)],
                            start=(t == 0), stop=(t == NT - 1),
                            skip_group_check=True,
                        )
                nc.vector.reciprocal_approx_fast(out=den, in_=denp)
                rb16 = attnp.tile([HEADS, HW], BF16, tag="recip16")
                nc.vector.tensor_copy(rb16, den)

                # recip broadcast head -> channels (PE expansion), copied to SBUF
                recip_bc = attnp.tile([P, PT, HW], F32, tag="recip_bc")
                for mc in range(PT):
                    ps = pmm.tile([P, HW], F32, tag="mm")
                    for n in range(NHALF):
                        nc.tensor.matmul(ps[:, _ns(n)], expm4[:HEADS, mc, :],
                                         rb16[:, _ns(n)], start=True, stop=True)
                    nc.vector.tensor_copy(recip_bc[:, mc, :], ps)

                # ---- v side: out_pre[c] = sum_kk e_bc * v_shift ----
                h2 = actp.tile([P, PT, HW], BF16, tag="h2")
                for mc in range(PT):
                    acc = paccp.tile([P, HW], F32, tag="acc")
                    for kk in range(NKK):
                        t, j = kk // 4, kk % 4
                        di, dj = kk // KS, kk % KS
                        eb = pmm.tile([P, HW], F32, tag="mm")
                        for n in range(NHALF):
                            nc.tensor.matmul(
                                eb[:, _ns(n)],
                                expm4[32 * j:32 * (j + 1), mc, :],
                                epks[t][32 * j:32 * (j + 1), _ns(n)],
                                start=True, stop=True,
                                tile_position=(32 * j, 0),
                            )
                        t2 = tmp2p.tile([P, HW], BF16, tag="tmp2")
                        nc.vector.tensor_tensor(
                            out=t2.rearrange("p (a b) -> p a b", a=H),
                            in0=eb.rearrange("p (a b) -> p a b", a=H),
                            in1=vpad[:, mc, di:di + H, dj:dj + W],
                            op=mybir.AluOpType.mult,
                        )
                        for n in range(NHALF):
                            nc.tensor.matmul(
                                acc[:, _ns(n)], ident, t2[:, _ns(n)],
                                start=(kk == 0), stop=(kk == NKK - 1),
                                skip_group_check=True,
                            )
                    # h2 = relu(acc * recip_bc + batt)
                    t3 = tmp2p.tile([P, HW], F32, tag="t3")
                    nc.vector.tensor_tensor(
                        out=t3, in0=acc, in1=recip_bc[:, mc, :],
                        op=mybir.AluOpType.mult,
                    )
                    nc.scalar.activation(
                        out=h2[:, mc, :], in_=t3,
                        func=mybir.ActivationFunctionType.Relu,
                        bias=batt[:, mc], scale=1.0,
                    )

                # ---- conv3 + residual + relu ----
                for oc in range(OC):
                    xr = xresp.tile([P, HW], F32, tag="xres")
                    nc.sync.dma_start(out=xr, in_=xf_d[b, oc])
                    ps = pmm.tile([P, HW], F32, tag="mm")
                    for kc in range(PT):
                        for n in range(NHALF):
                            nc.tensor.matmul(
                                ps[:, _ns(n)],
                                w3T[:, kc, oc * P:(oc + 1) * P],
                                h2[:, kc, _ns(n)],
                                start=(kc == 0), stop=(kc == PT - 1),
                            )
                    z = outzp.tile([P, HW], F32, tag="outz")
                    # z = (x + b3) + psum
                    nc.vector.scalar_tensor_tensor(
                        out=z, in0=xr, scalar=b3[:, oc], in1=ps,
                        op0=mybir.AluOpType.add, op1=mybir.AluOpType.add,
                    )
                    zr = outzp.tile([P, HW], F32, tag="outzr")
                    nc.scalar.activation(
                        out=zr, in_=z, func=mybir.ActivationFunctionType.Relu,
                    )
                    nc.sync.dma_start(out=out_d[b, oc], in_=zr)

    nc.compile()
    return nc


_PROG = None


def _host_prep(inputs):
    import ml_dtypes
    bf = ml_dtypes.bfloat16
    f = lambda a: np.asarray(a, dtype=np.float32)
    x = f(inputs["x"])
    # fold bn scales into weights (bn(conv(x,W),s,b) = conv(x, s*W) + b)
    w1 = f(inputs["w_conv1"]) * f(inputs["bn1_s"])[:, None]
    wq = f(inputs["wq"]) * f(inputs["bnq_s"])[:, None]
    wk = f(inputs["wk"]) * f(inputs["bnk_s"])[:, None]
    # fold bnatt scale through the (linear) attention-value path into v
    sv = f(inputs["bnatt_s"]) * f(inputs["bnv_s"])
    wv = f(inputs["wv"]) * sv[:, None]
    bv = f(inputs["bnatt_s"]) * f(inputs["bnv_b"])
    w3 = f(inputs["w_conv3"]) * f(inputs["bn3_s"])[:, None]

    posf = (f(inputs["pos_h"]) + f(inputs["pos_w"])).reshape(WIDTH, NKK)

    sel = np.zeros((PT, P, HEADS), np.float32)
    for pt in range(P // 64):
        pass
    for pt in range(PT):
        for c in range(P):
            sel[pt, c, pt * (P // D) + c // D] = 1.0
    # p2[pt, c, t, 32*j+g] = pos[c_global, 4t+j] if head(c_global)==g
    p2 = np.zeros((PT, P, NT, P), np.float32)
    for pt in range(PT):
        for c in range(P):
            g = pt * (P // D) + c // D
            for kk in range(NKK):
                t, j = kk // 4, kk % 4
                p2[pt, c, t, 32 * j + g] = posf[pt * P + c, kk]
    # sab[r, g] = 1 if r % 32 == g (sum over the 4 packed kk rows)
    sab = np.zeros((P, HEADS), np.float32)
    for r in range(P):
        sab[r, r % HEADS] = 1.0
    # expm4[32j+g, mc, c] = 1 if head(mc*128+c) == g   (replicated per row strip)
    expm4 = np.zeros((P, PT, P), np.float32)
    for jj in range(4):
        for g in range(HEADS):
            for mc in range(PT):
                for c in range(P):
                    if mc * (P // D) + c // D == g:
                        expm4[32 * jj + g, mc, c] = 1.0

    com = {
        "w1T": np.ascontiguousarray(w1.T.reshape(KC1, P, WIDTH)).astype(bf),
        "wqT": np.ascontiguousarray(wq.T.reshape(PT, P, WIDTH)).astype(bf),
        "wkT": np.ascontiguousarray(wk.T.reshape(PT, P, WIDTH)).astype(bf),
        "wvT": np.ascontiguousarray(wv.T.reshape(PT, P, WIDTH)).astype(bf),
        "w3T": np.ascontiguousarray(w3.T.reshape(PT, P, OUT)).astype(bf),
        "b1": f(inputs["bn1_b"]).reshape(PT, P, 1),
        "bq": f(inputs["bnq_b"]).reshape(PT, P, 1),
        "bk": f(inputs["bnk_b"]).reshape(PT, P, 1),
        "bv": bv.reshape(PT, P, 1),
        "batt": f(inputs["bnatt_b"]).reshape(PT, P, 1),
        "b3": f(inputs["bn3_b"]).reshape(OC, P, 1),
        "sel": sel.astype(bf),
        "p2": p2.astype(bf),
        "sab": sab.astype(bf),
        "eye32": np.eye(HEADS, dtype=np.float32).astype(bf),
        "expm4": expm4.astype(bf),
        "ident": np.eye(P, dtype=np.float32).astype(bf),
    }
    xr = x.reshape(B, KC1, P, HW)
    in_maps = []
    for c in range(NC_):
        xs = np.ascontiguousarray(xr[c * BL:(c + 1) * BL])
        in_maps.append(dict(com, x16=xs.astype(bf), xf=xs))
    return in_maps


def kernel(**inputs):
    global _PROG
    if _PROG is None:
        _PROG = build_program()
    in_maps = _host_prep(inputs)
    res = run_bass_kernel_spmd(_PROG, in_maps, core_ids=list(range(NC_)))
    outs = [res.results[c]["out"].reshape(BL, OUT, H, W) for c in range(NC_)]
    return np.concatenate(outs, axis=0)


# revision 31
# speedup vs baseline: 3.2481x; 1.1165x over previous
"""Trainium2 Bass kernel for nn_Bottleneck_75325136437765 (sparse 3x3 local attention bottleneck).

Sharding: data-parallel over batch B=16 across 8 cores (2 batches/core), params replicated.

Per-core layout: channels on partitions, spatial (32*32=1024) on free dim. All matmuls bf16
(fp32 matmuls cost 2 PE passes on TRN2), fp32 PSUM accumulation everywhere.

  conv1/qkv/conv3: plain matmuls (lhsT = transposed weights, host-precomputed, bn scales folded).
  attention logits, packed PSUM layout (row = 32*(kk%4) + head, 3 tiles of 4 shifts):
      L[g,kk,hw] = sum_d q[gd,hw]*k[gd,hw+off_kk]  (col-tiled 0/1-selection matmuls over products)
                 + sum_d q[gd,hw]*pos[gd,kk]       (P2 matmul, accumulated into same PSUM)
  softmax over kk without max-subtraction, 1/sum factored out to the end:
      e = exp(L) (packed, 3 ACT ops); den = sum_kk e via 0/1 matmuls; recip = 1/den
      out_pre[c,hw] = sum_kk e_bc[c,kk,hw] * v[c,hw+off_kk]
        e_bc via row-tiled expansion matmuls; per-shift product on DVE;
        sum over kk via identity-matmul PSUM accumulation
      h2 = relu(out_pre * recip_bc + bnatt_b)
  residual: x streamed again in fp32, fused (x+b3)+psum on DVE, relu on ACT.
"""

import numpy as np

import concourse.bass as bass
import concourse.bacc as bacc
import concourse.tile as tile
from concourse import mybir
from concourse.bass_utils import run_bass_kernel_spmd

# ---- problem constants (hardcoded per contract) ----
B, CIN, H, W = 16, 1024, 32, 32
WIDTH, OUT, HEADS, KS = 256, 1024, 32, 3
D = WIDTH // HEADS            # 8 channels per head
HW = H * W                    # 1024
NC_ = 8                       # cores
BL = B // NC_                 # 2 batches per core
P = 128
KC1 = CIN // P                # 8 contraction chunks for conv1
PT = WIDTH // P               # 2 partition tiles for width-256 tensors
OC = OUT // P                 # 8 output ptiles for conv3
NKK = KS * KS                 # 9 shifts
NT = 3                        # packed logit tiles (4+4+1 shifts)
F32 = mybir.dt.float32
BF16 = mybir.dt.bfloat16
NHALF = 2                     # PSUM-bank limit: matmul N<=512 fp32 out


def _ns(n):
    return slice(n * 512, (n + 1) * 512)


def build_program():
    nc = bacc.Bacc(None, target_bir_lowering=False, debug=False)

    def din(name, shape, dt=BF16):
        return nc.dram_tensor(name, list(shape), dt, kind="ExternalInput").ap()

    x16_d = din("x16", (BL, KC1, P, HW))
    w1T_d = din("w1T", (KC1, P, WIDTH))
    wqT_d = din("wqT", (PT, P, WIDTH))
    wkT_d = din("wkT", (PT, P, WIDTH))
    wvT_d = din("wvT", (PT, P, WIDTH))
    w3T_d = din("w3T", (PT, P, OUT))
    b1_d = din("b1", (PT, P, 1), F32)
    bq_d = din("bq", (PT, P, 1), F32)
    bk_d = din("bk", (PT, P, 1), F32)
    bv_d = din("bv", (PT, P, 1), F32)
    batt_d = din("batt", (PT, P, 1), F32)
    b3_d = din("b3", (OC, P, 1), F32)
    sel_d = din("sel", (PT, P, HEADS))
    p2_d = din("p2", (PT, P, NT, P))
    sab_d = din("sab", (P, HEADS))
    eye32_d = din("eye32", (HEADS, HEADS))
    expm4_d = din("expm4", (P, PT, P))
    ident_d = din("ident", (P, P))
    out_d = nc.dram_tensor("out", [BL, OC, P, HW], F32, kind="ExternalOutput").ap()

    with tile.TileContext(nc) as tc:
        with (
            tc.tile_pool(name="consts", bufs=1) as consts,
            tc.tile_pool(name="xb", bufs=2) as xbp,
            tc.tile_pool(name="act", bufs=2) as actp,
            tc.tile_pool(name="attn", bufs=1) as attnp,
            tc.tile_pool(name="epk", bufs=4) as epkp,
            tc.tile_pool(name="tmp", bufs=10) as tmpp,
            tc.tile_pool(name="tmp2", bufs=4) as tmp2p,
            tc.tile_pool(name="outz", bufs=3) as outzp,
            tc.tile_pool(name="pmm", bufs=2, space="PSUM") as pmm,
            tc.tile_pool(name="pL", bufs=1, space="PSUM") as pLp,
            tc.tile_pool(name="pacc", bufs=1, space="PSUM") as paccp,
        ):
            # ---- load constants ----
            # constants other than w1T/b1 go on the SWDGE queue so the sync
            # queue serves conv1's x/w chunks first (fast kernel start)
            def cload(name, dram, shape, dt=BF16, re="k p m -> p k m"):
                t = consts.tile(shape, dt, tag=name)
                nc.gpsimd.dma_start(out=t, in_=dram.rearrange(re) if re else dram)
                return t

            w1T = consts.tile([P, KC1, WIDTH], BF16, tag="w1T")
            for kc in range(KC1):
                nc.sync.dma_start(out=w1T[:, kc, :], in_=w1T_d[kc])
            b1 = consts.tile([P, PT, 1], F32, tag="b1")
            nc.sync.dma_start(out=b1, in_=b1_d.rearrange("k p m -> p k m"))
            wqT = cload("wqT", wqT_d, [P, PT, WIDTH])
            wkT = cload("wkT", wkT_d, [P, PT, WIDTH])
            wvT = cload("wvT", wvT_d, [P, PT, WIDTH])
            w3T = cload("w3T", w3T_d, [P, PT, OUT])
            bq = cload("bq", bq_d, [P, PT, 1], F32)
            bk = cload("bk", bk_d, [P, PT, 1], F32)
            bv = cload("bv", bv_d, [P, PT, 1], F32)
            batt = cload("batt", batt_d, [P, PT, 1], F32)
            b3 = cload("b3", b3_d, [P, OC, 1], F32)
            sel = cload("sel", sel_d, [P, PT, HEADS])
            p2 = cload("p2", p2_d, [P, PT, NT, P], re="k p m o -> p k m o")
            sab = cload("sab", sab_d, [P, HEADS], re=None)
            eye32 = cload("eye32", eye32_d, [HEADS, HEADS], re=None)
            expm4 = cload("expm4", expm4_d, [P, PT, P], re=None)
            ident = cload("ident", ident_d, [P, P], re=None)

            # persistent zero-padded k/v tiles (borders stay zero across batches)
            kpad = consts.tile([P, PT, H + 2, W + 2], BF16)
            vpad = consts.tile([P, PT, H + 2, W + 2], BF16)
            nc.vector.memset(kpad, 0.0)
            nc.vector.memset(vpad, 0.0)

            for b in range(BL):
                # ---- load x (bf16 for conv1), chunked so conv1 starts early ----
                xb = xbp.tile([P, KC1, HW], BF16, tag="xb")
                for kc in range(KC1):
                    nc.sync.dma_start(out=xb[:, kc, :], in_=x16_d[b, kc])

                # ---- conv1: h1 = relu(x @ w1' + b1) ----
                h1 = actp.tile([P, PT, HW], BF16, tag="h1")
                for mc in range(PT):
                    ps = pmm.tile([P, HW], F32, tag="mm")
                    for kc in range(KC1):
                        for n in range(NHALF):
                            nc.tensor.matmul(
                                ps[:, _ns(n)],
                                w1T[:, kc, mc * P:(mc + 1) * P],
                                xb[:, kc, _ns(n)],
                                start=(kc == 0), stop=(kc == KC1 - 1),
                            )
                    nc.scalar.activation(
                        out=h1[:, mc, :], in_=ps,
                        func=mybir.ActivationFunctionType.Relu,
                        bias=b1[:, mc], scale=1.0,
                    )

                # ---- q/k/v convs ----
                q = actp.tile([P, PT, HW], BF16, tag="q")
                for wT, bias, relu, dest in (
                    (wqT, bq, True, None),
                    (wkT, bk, True, kpad),
                    (wvT, bv, False, vpad),
                ):
                    for mc in range(PT):
                        ps = pmm.tile([P, HW], F32, tag="mm")
                        for kc in range(PT):
                            for n in range(NHALF):
                                nc.tensor.matmul(
                                    ps[:, _ns(n)],
                                    wT[:, kc, mc * P:(mc + 1) * P],
                                    h1[:, kc, _ns(n)],
                                    start=(kc == 0), stop=(kc == PT - 1),
                                )
                        if dest is None:
                            o, i = q[:, mc, :], ps[:]
                        else:
                            o = dest[:, mc, 1:H + 1, 1:W + 1]
                            i = ps.rearrange("p (a b) -> p a b", a=H)
                        nc.scalar.activation(
                            out=o, in_=i,
                            func=(mybir.ActivationFunctionType.Relu if relu
                                  else mybir.ActivationFunctionType.Identity),
                            bias=bias[:, mc], scale=1.0,
                        )


                # ---- attention logits (packed), exp, denominator ----
                # packed tile t rows: 32*(kk%4) + g  for kk in {4t..4t+3}
                epks = []
                den = attnp.tile([HEADS, HW], F32, tag="den")
                denp = paccp.tile([HEADS, HW], F32, tag="acc")
                for t in range(NT):
                    nsh = 4 if t < 2 else 1
                    rows = 32 * nsh
                    Lpk = pLp.tile([P, HW], F32, tag="Lpk")
                    # qpos term: all rows at once per pt chunk
                    for n in range(NHALF):
                        for pt in range(PT):
                            nc.tensor.matmul(
                                Lpk[:rows, _ns(n)],
                                p2[:, pt, t, :rows],
                                q[:, pt, _ns(n)],
                                start=(pt == 0), stop=False,
                                skip_group_check=True,
                            )
                    # qk products + col-tiled group reduce
                    for j in range(nsh):
                        kk = 4 * t + j
                        di, dj = kk // KS, kk % KS
                        for pt in range(PT):
                            tmp = tmpp.tile([P, HW], BF16, tag="tmp")
                            nc.vector.tensor_tensor(
                                out=tmp.rearrange("p (a b) -> p a b", a=H),
                                in0=kpad[:, pt, di:di + H, dj:dj + W],
                                in1=q[:, pt, :].rearrange("p (a b) -> p a b", a=H),
                                op=mybir.AluOpType.mult,
                            )
                            for n in range(NHALF):
                                nc.tensor.matmul(
                                    Lpk[32 * j:32 * (j + 1), _ns(n)],
                                    sel[:, pt, :],
                                    tmp[:, _ns(n)],
                                    start=False, stop=(pt == PT - 1),
                                    tile_position=(0, 32 * j),
                                    skip_group_check=True,
                                )
                    epk = epkp.tile([P, HW], BF16, tag="epk")
                    nc.scalar.activation(
                        out=epk[:rows, :], in_=Lpk[:rows, :],
                        func=mybir.ActivationFunctionType.Exp,
                    )
                    epks.append(epk)
                    # denominator accumulation
                    lhs = sab if t < 2 else eye32
                    for n in range(NHALF):
                        nc.tensor.matmul(
                            denp[:, _ns(n)], lhs[:rows, :], epk[:rows, _ns(n)],
                            start=(t == 0), stop=(t == NT - 1),
                            skip_group_check=True,
                        )
                nc.vector.reciprocal_approx_fast(out=den, in_=denp)
                rb16 = attnp.tile([HEADS, HW], BF16, tag="recip16")
                nc.vector.tensor_copy(rb16, den)

                # recip broadcast head -> channels (PE expansion), copied to SBUF
                recip_bc = attnp.tile([P, PT, HW], F32, tag="recip_bc")
                for mc in range(PT):
                    ps = pmm.tile([P, HW], F32, tag="mm")
                    for n in range(NHALF):
                        nc.tensor.matmul(ps[:, _ns(n)], expm4[:HEADS, mc, :],
                                         rb16[:, _ns(n)], start=True, stop=True)
                    nc.vector.tensor_copy(recip_bc[:, mc, :], ps)

                # ---- v side: out_pre[c] = sum_kk e_bc * v_shift ----
                h2 = actp.tile([P, PT, HW], BF16, tag="h2")
                for mc in range(PT):
                    acc = paccp.tile([P, HW], F32, tag="acc")
                    for kk in range(NKK):
                        t, j = kk // 4, kk % 4
                        di, dj = kk // KS, kk % KS
                        eb = pmm.tile([P, HW], F32, tag="mm")
                        for n in range(NHALF):
                            nc.tensor.matmul(
                                eb[:, _ns(n)],
                                expm4[32 * j:32 * (j + 1), mc, :],
                                epks[t][32 * j:32 * (j + 1), _ns(n)],
                                start=True, stop=True,
                                tile_position=(32 * j, 0),
                            )
                        t2 = tmp2p.tile([P, HW], BF16, tag="tmp2")
                        nc.vector.tensor_tensor(
                            out=t2.rearrange("p (a b) -> p a b", a=H),
                            in0=eb.rearrange("p (a b) -> p a b", a=H),
                            in1=vpad[:, mc, di:di + H, dj:dj + W],
                            op=mybir.AluOpType.mult,
                        )
                        for n in range(NHALF):
                            nc.tensor.matmul(
                                acc[:, _ns(n)], ident, t2[:, _ns(n)],
                                start=(kk == 0), stop=(kk == NKK - 1),
                                skip_group_check=True,
                            )
                    # h2 = relu(acc * recip_bc + batt)
                    t3 = tmp2p.tile([P, HW], F32, tag="t3")
                    nc.vector.tensor_tensor(
                        out=t3, in0=acc, in1=recip_bc[:, mc, :],
                        op=mybir.AluOpType.mult,
                    )
                    nc.scalar.activation(
                        out=h2[:, mc, :], in_=t3,
                        func=mybir.ActivationFunctionType.Relu,
                        bias=batt[:, mc], scale=1.0,
                    )

                # ---- conv3 + residual (identity matmul on bf16 x) + relu ----
                for oc in range(OC):
                    ps = pmm.tile([P, HW], F32, tag="mm")
                    for n in range(NHALF):
                        for kc in range(PT):
                            nc.tensor.matmul(
                                ps[:, _ns(n)],
                                w3T[:, kc, oc * P:(oc + 1) * P],
                                h2[:, kc, _ns(n)],
                                start=(kc == 0), stop=False,
                                skip_group_check=True,
                            )
                        nc.tensor.matmul(
                            ps[:, _ns(n)], ident, xb[:, oc, _ns(n)],
                            start=False, stop=True,
                            skip_group_check=True,
                        )
                    zr = outzp.tile([P, HW], F32, tag="outzr")
                    nc.scalar.activation(
                        out=zr, in_=ps, func=mybir.ActivationFunctionType.Relu,
                        bias=b3[:, oc], scale=1.0,
                    )
                    nc.sync.dma_start(out=out_d[b, oc], in_=zr)

    nc.compile()
    return nc


_PROG = None


def _host_prep(inputs):
    import ml_dtypes
    bf = ml_dtypes.bfloat16
    f = lambda a: np.asarray(a, dtype=np.float32)
    x = f(inputs["x"])
    # fold bn scales into weights (bn(conv(x,W),s,b) = conv(x, s*W) + b)
    w1 = f(inputs["w_conv1"]) * f(inputs["bn1_s"])[:, None]
    wq = f(inputs["wq"]) * f(inputs["bnq_s"])[:, None]
    wk = f(inputs["wk"]) * f(inputs["bnk_s"])[:, None]
    # fold bnatt scale through the (linear) attention-value path into v
    sv = f(inputs["bnatt_s"]) * f(inputs["bnv_s"])
    wv = f(inputs["wv"]) * sv[:, None]
    bv = f(inputs["bnatt_s"]) * f(inputs["bnv_b"])
    w3 = f(inputs["w_conv3"]) * f(inputs["bn3_s"])[:, None]

    posf = (f(inputs["pos_h"]) + f(inputs["pos_w"])).reshape(WIDTH, NKK)

    sel = np.zeros((PT, P, HEADS), np.float32)
    for pt in range(P // 64):
        pass
    for pt in range(PT):
        for c in range(P):
            sel[pt, c, pt * (P // D) + c // D] = 1.0
    # p2[pt, c, t, 32*j+g] = pos[c_global, 4t+j] if head(c_global)==g
    p2 = np.zeros((PT, P, NT, P), np.float32)
    for pt in range(PT):
        for c in range(P):
            g = pt * (P // D) + c // D
            for kk in range(NKK):
                t, j = kk // 4, kk % 4
                p2[pt, c, t, 32 * j + g] = posf[pt * P + c, kk]
    # sab[r, g] = 1 if r % 32 == g (sum over the 4 packed kk rows)
    sab = np.zeros((P, HEADS), np.float32)
    for r in range(P):
        sab[r, r % HEADS] = 1.0
    # expm4[32j+g, mc, c] = 1 if head(mc*128+c) == g   (replicated per row strip)
    expm4 = np.zeros((P, PT, P), np.float32)
    for jj in range(4):
        for g in range(HEADS):
            for mc in range(PT):
                for c in range(P):
                    if mc * (P // D) + c // D == g:
                        expm4[32 * jj + g, mc, c] = 1.0

    com = {
        "w1T": np.ascontiguousarray(w1.T.reshape(KC1, P, WIDTH)).astype(bf),
        "wqT": np.ascontiguousarray(wq.T.reshape(PT, P, WIDTH)).astype(bf),
        "wkT": np.ascontiguousarray(wk.T.reshape(PT, P, WIDTH)).astype(bf),
        "wvT": np.ascontiguousarray(wv.T.reshape(PT, P, WIDTH)).astype(bf),
        "w3T": np.ascontiguousarray(w3.T.reshape(PT, P, OUT)).astype(bf),
        "b1": f(inputs["bn1_b"]).reshape(PT, P, 1),
        "bq": f(inputs["bnq_b"]).reshape(PT, P, 1),
        "bk": f(inputs["bnk_b"]).reshape(PT, P, 1),
        "bv": bv.reshape(PT, P, 1),
        "batt": f(inputs["bnatt_b"]).reshape(PT, P, 1),
        "b3": f(inputs["bn3_b"]).reshape(OC, P, 1),
        "sel": sel.astype(bf),
        "p2": p2.astype(bf),
        "sab": sab.astype(bf),
        "eye32": np.eye(HEADS, dtype=np.float32).astype(bf),
        "expm4": expm4.astype(bf),
        "ident": np.eye(P, dtype=np.float32).astype(bf),
    }
    xr = x.reshape(B, KC1, P, HW)
    in_maps = []
    for c in range(NC_):
        xs = np.ascontiguousarray(xr[c * BL:(c + 1) * BL])
        in_maps.append(dict(com, x16=xs.astype(bf)))
    return in_maps


def kernel(**inputs):
    global _PROG
    if _PROG is None:
        _PROG = build_program()
    in_maps = _host_prep(inputs)
    res = run_bass_kernel_spmd(_PROG, in_maps, core_ids=list(range(NC_)))
    outs = [res.results[c]["out"].reshape(BL, OUT, H, W) for c in range(NC_)]
    return np.concatenate(outs, axis=0)


# revision 36
# speedup vs baseline: 3.3770x; 1.0397x over previous
"""Trainium2 Bass kernel for nn_Bottleneck_75325136437765 (sparse 3x3 local attention bottleneck).

Sharding: data-parallel over batch B=16 across 8 cores (2 batches/core), params replicated.

Per-core layout: channels on partitions, spatial (32*32=1024) on free dim. All matmuls bf16
(fp32 matmuls cost 2 PE passes on TRN2), fp32 PSUM accumulation everywhere.

  conv1/qkv/conv3: plain matmuls (lhsT = transposed weights, host-precomputed, bn scales folded).
  attention logits, packed PSUM layout (row = 32*(kk%4) + head, 3 tiles of 4 shifts):
      L[g,kk,hw] = sum_d q[gd,hw]*k[gd,hw+off_kk]  (col-tiled 0/1-selection matmuls over products)
                 + sum_d q[gd,hw]*pos[gd,kk]       (P2 matmul, accumulated into same PSUM)
  softmax over kk without max-subtraction, 1/sum factored out to the end:
      e = exp(L) (packed, 3 ACT ops); den = sum_kk e via 0/1 matmuls; recip = 1/den
      out_pre[c,hw] = sum_kk e_bc[c,kk,hw] * v[c,hw+off_kk]
        e_bc via row-tiled expansion matmuls; per-shift product on DVE;
        sum over kk via identity-matmul PSUM accumulation
      h2 = relu(out_pre * recip_bc + bnatt_b)
  residual: x streamed again in fp32, fused (x+b3)+psum on DVE, relu on ACT.
"""

import numpy as np

import concourse.bass as bass
import concourse.bacc as bacc
import concourse.tile as tile
from concourse import mybir
from concourse.bass_utils import run_bass_kernel_spmd

# ---- problem constants (hardcoded per contract) ----
B, CIN, H, W = 16, 1024, 32, 32
WIDTH, OUT, HEADS, KS = 256, 1024, 32, 3
D = WIDTH // HEADS            # 8 channels per head
HW = H * W                    # 1024
NC_ = 8                       # cores
BL = B // NC_                 # 2 batches per core
P = 128
KC1 = CIN // P                # 8 contraction chunks for conv1
PT = WIDTH // P               # 2 partition tiles for width-256 tensors
OC = OUT // P                 # 8 output ptiles for conv3
NKK = KS * KS                 # 9 shifts
NT = 3                        # packed logit tiles (4+4+1 shifts)
F32 = mybir.dt.float32
BF16 = mybir.dt.bfloat16
NHALF = 2                     # PSUM-bank limit: matmul N<=512 fp32 out


def _ns(n):
    return slice(n * 512, (n + 1) * 512)


def build_program():
    nc = bacc.Bacc(None, target_bir_lowering=False, debug=False)

    def din(name, shape, dt=BF16):
        return nc.dram_tensor(name, list(shape), dt, kind="ExternalInput").ap()

    x16_d = din("x16", (BL, KC1, P, HW))
    w1T_d = din("w1T", (KC1, P, WIDTH))
    wqT_d = din("wqT", (PT, P, WIDTH))
    wkT_d = din("wkT", (PT, P, WIDTH))
    wvT_d = din("wvT", (PT, P, WIDTH))
    w3T_d = din("w3T", (PT, P, OUT))
    b1_d = din("b1", (PT, P, 1), F32)
    bq_d = din("bq", (PT, P, 1), F32)
    bk_d = din("bk", (PT, P, 1), F32)
    bv_d = din("bv", (PT, P, 1), F32)
    batt_d = din("batt", (PT, P, 1), F32)
    b3_d = din("b3", (OC, P, 1), F32)
    sel_d = din("sel", (PT, P, HEADS))
    p2_d = din("p2", (PT, P, NT, P))
    sab_d = din("sab", (P, HEADS))
    eye32_d = din("eye32", (HEADS, HEADS))
    ident_d = din("ident", (P, P))
    out_d = nc.dram_tensor("out", [BL, OC, P, HW], F32, kind="ExternalOutput").ap()

    with tile.TileContext(nc) as tc:
        with (
            tc.tile_pool(name="consts", bufs=1) as consts,
            tc.tile_pool(name="xb", bufs=2) as xbp,
            tc.tile_pool(name="act", bufs=2) as actp,
            tc.tile_pool(name="attn", bufs=1) as attnp,
            tc.tile_pool(name="epk", bufs=4) as epkp,
            tc.tile_pool(name="tmp", bufs=10) as tmpp,
            tc.tile_pool(name="tmp2", bufs=4) as tmp2p,
            tc.tile_pool(name="outz", bufs=3) as outzp,
            tc.tile_pool(name="pmm", bufs=2, space="PSUM") as pmm,
            tc.tile_pool(name="pL", bufs=1, space="PSUM") as pLp,
            tc.tile_pool(name="pacc", bufs=1, space="PSUM") as paccp,
        ):
            # ---- load constants ----
            # constants other than w1T/b1 go on the SWDGE queue so the sync
            # queue serves conv1's x/w chunks first (fast kernel start)
            def cload(name, dram, shape, dt=BF16, re="k p m -> p k m"):
                t = consts.tile(shape, dt, tag=name)
                nc.gpsimd.dma_start(out=t, in_=dram.rearrange(re) if re else dram)
                return t

            w1T = consts.tile([P, KC1, WIDTH], BF16, tag="w1T")
            b1 = consts.tile([P, PT, 1], F32, tag="b1")
            nc.sync.dma_start(out=b1, in_=b1_d.rearrange("k p m -> p k m"))
            wqT = cload("wqT", wqT_d, [P, PT, WIDTH])
            wkT = cload("wkT", wkT_d, [P, PT, WIDTH])
            wvT = cload("wvT", wvT_d, [P, PT, WIDTH])
            w3T = cload("w3T", w3T_d, [P, PT, OUT])
            bq = cload("bq", bq_d, [P, PT, 1], F32)
            bk = cload("bk", bk_d, [P, PT, 1], F32)
            bv = cload("bv", bv_d, [P, PT, 1], F32)
            batt = cload("batt", batt_d, [P, PT, 1], F32)
            b3 = cload("b3", b3_d, [P, OC, 1], F32)
            sel = cload("sel", sel_d, [P, PT, HEADS])
            p2 = cload("p2", p2_d, [P, PT, NT, P], re="k p m o -> p k m o")
            sab = cload("sab", sab_d, [P, HEADS], re=None)
            eye32 = cload("eye32", eye32_d, [HEADS, HEADS], re=None)
            ident = cload("ident", ident_d, [P, P], re=None)

            def head_bcast_dma(dst, src16):
                # dst[g*8+d, :] = src16[g, :] — 2-level partition AP broadcast
                bc = bass.AP(tensor=src16.tensor, offset=src16.offset,
                             ap=[list(src16.ap[0]), [0, D]]
                                + [list(a) for a in src16.ap[1:]])
                nc.sync.dma_start(out=dst, in_=bc)

            # persistent zero-padded k/v tiles (borders stay zero across batches)
            kpad = consts.tile([P, PT, H + 2, W + 2], BF16)
            vpad = consts.tile([P, PT, H + 2, W + 2], BF16)
            nc.vector.memset(kpad, 0.0)
            nc.vector.memset(vpad, 0.0)

            for b in range(BL):
                # ---- load x (bf16 for conv1), chunked so conv1 starts early ----
                xb = xbp.tile([P, KC1, HW], BF16, tag="xb")
                for kc in range(KC1):
                    if b == 0:
                        nc.sync.dma_start(out=w1T[:, kc, :], in_=w1T_d[kc])
                    nc.sync.dma_start(out=xb[:, kc, :], in_=x16_d[b, kc])

                # ---- conv1: h1 = relu(x @ w1' + b1) ----
                h1 = actp.tile([P, PT, HW], BF16, tag="h1")
                for mc in range(PT):
                    ps = pmm.tile([P, HW], F32, tag="mm")
                    for kc in range(KC1):
                        for n in range(NHALF):
                            nc.tensor.matmul(
                                ps[:, _ns(n)],
                                w1T[:, kc, mc * P:(mc + 1) * P],
                                xb[:, kc, _ns(n)],
                                start=(kc == 0), stop=(kc == KC1 - 1),
                            )
                    nc.scalar.activation(
                        out=h1[:, mc, :], in_=ps,
                        func=mybir.ActivationFunctionType.Relu,
                        bias=b1[:, mc], scale=1.0,
                    )

                # ---- q/k/v convs ----
                q = actp.tile([P, PT, HW], BF16, tag="q")
                for wT, bias, relu, dest in (
                    (wqT, bq, True, None),
                    (wkT, bk, True, kpad),
                    (wvT, bv, False, vpad),
                ):
                    for mc in range(PT):
                        ps = pmm.tile([P, HW], F32, tag="mm")
                        for kc in range(PT):
                            for n in range(NHALF):
                                nc.tensor.matmul(
                                    ps[:, _ns(n)],
                                    wT[:, kc, mc * P:(mc + 1) * P],
                                    h1[:, kc, _ns(n)],
                                    start=(kc == 0), stop=(kc == PT - 1),
                                )
                        if dest is None:
                            o, i = q[:, mc, :], ps[:]
                        else:
                            o = dest[:, mc, 1:H + 1, 1:W + 1]
                            i = ps.rearrange("p (a b) -> p a b", a=H)
                        nc.scalar.activation(
                            out=o, in_=i,
                            func=(mybir.ActivationFunctionType.Relu if relu
                                  else mybir.ActivationFunctionType.Identity),
                            bias=bias[:, mc], scale=1.0,
                        )


                # ---- attention logits (packed), exp, denominator ----
                # packed tile t rows: 32*(kk%4) + g  for kk in {4t..4t+3}
                epks = []
                den = attnp.tile([HEADS, HW], F32, tag="den")
                denp = paccp.tile([HEADS, HW], F32, tag="acc")
                for t in range(NT):
                    nsh = 4 if t < 2 else 1
                    rows = 32 * nsh
                    Lpk = pLp.tile([P, HW], F32, tag="Lpk")
                    # qpos term: all rows at once per pt chunk
                    for n in range(NHALF):
                        for pt in range(PT):
                            nc.tensor.matmul(
                                Lpk[:rows, _ns(n)],
                                p2[:, pt, t, :rows],
                                q[:, pt, _ns(n)],
                                start=(pt == 0), stop=False,
                                skip_group_check=True,
                            )
                    # qk products + col-tiled group reduce
                    for j in range(nsh):
                        kk = 4 * t + j
                        di, dj = kk // KS, kk % KS
                        for pt in range(PT):
                            tmp = tmpp.tile([P, HW], BF16, tag="tmp")
                            nc.vector.tensor_tensor(
                                out=tmp.rearrange("p (a b) -> p a b", a=H),
                                in0=kpad[:, pt, di:di + H, dj:dj + W],
                                in1=q[:, pt, :].rearrange("p (a b) -> p a b", a=H),
                                op=mybir.AluOpType.mult,
                            )
                            for n in range(NHALF):
                                nc.tensor.matmul(
                                    Lpk[32 * j:32 * (j + 1), _ns(n)],
                                    sel[:, pt, :],
                                    tmp[:, _ns(n)],
                                    start=False, stop=(pt == PT - 1),
                                    tile_position=(0, 32 * j),
                                    skip_group_check=True,
                                )
                    epk = epkp.tile([P, HW], BF16, tag="epk")
                    nc.scalar.activation(
                        out=epk[:rows, :], in_=Lpk[:rows, :],
                        func=mybir.ActivationFunctionType.Exp,
                    )
                    epks.append(epk)
                    # denominator accumulation
                    lhs = sab if t < 2 else eye32
                    for n in range(NHALF):
                        nc.tensor.matmul(
                            denp[:, _ns(n)], lhs[:rows, :], epk[:rows, _ns(n)],
                            start=(t == 0), stop=(t == NT - 1),
                            skip_group_check=True,
                        )
                nc.vector.reciprocal_approx_fast(out=den, in_=denp)

                # recip broadcast head -> channels via DMA
                recip_bc = attnp.tile([P, PT, HW], F32, tag="recip_bc")
                for mc in range(PT):
                    head_bcast_dma(recip_bc[:, mc, :], den[16 * mc:16 * (mc + 1), :])

                # ---- v side: out_pre[c] = sum_kk e_bc * v_shift ----
                h2 = actp.tile([P, PT, HW], BF16, tag="h2")
                for mc in range(PT):
                    acc = paccp.tile([P, HW], F32, tag="acc")
                    for kk in range(NKK):
                        t, j = kk // 4, kk % 4
                        di, dj = kk // KS, kk % KS
                        eb = tmp2p.tile([P, HW], BF16, tag="ebc")
                        r0 = 32 * j + 16 * mc
                        head_bcast_dma(eb, epks[t][r0:r0 + 16, :])
                        t2 = tmp2p.tile([P, HW], BF16, tag="tmp2")
                        nc.vector.tensor_tensor(
                            out=t2.rearrange("p (a b) -> p a b", a=H),
                            in0=eb.rearrange("p (a b) -> p a b", a=H),
                            in1=vpad[:, mc, di:di + H, dj:dj + W],
                            op=mybir.AluOpType.mult,
                        )
                        for n in range(NHALF):
                            nc.tensor.matmul(
                                acc[:, _ns(n)], ident, t2[:, _ns(n)],
                                start=(kk == 0), stop=(kk == NKK - 1),
                                skip_group_check=True,
                            )
                    # h2 = relu(acc * recip_bc + batt)
                    t3 = tmp2p.tile([P, HW], F32, tag="t3")
                    nc.vector.tensor_tensor(
                        out=t3, in0=acc, in1=recip_bc[:, mc, :],
                        op=mybir.AluOpType.mult,
                    )
                    nc.scalar.activation(
                        out=h2[:, mc, :], in_=t3,
                        func=mybir.ActivationFunctionType.Relu,
                        bias=batt[:, mc], scale=1.0,
                    )

                # ---- conv3 + residual (identity matmul on bf16 x) + relu ----
                for oc in range(OC):
                    ps = pmm.tile([P, HW], F32, tag="mm")
                    for n in range(NHALF):
                        for kc in range(PT):
                            nc.tensor.matmul(
                                ps[:, _ns(n)],
                                w3T[:, kc, oc * P:(oc + 1) * P],
                                h2[:, kc, _ns(n)],
                                start=(kc == 0), stop=False,
                                skip_group_check=True,
                            )
                        nc.tensor.matmul(
                            ps[:, _ns(n)], ident, xb[:, oc, _ns(n)],
                            start=False, stop=True,
                            skip_group_check=True,
                        )
                    zr = outzp.tile([P, HW], F32, tag="outzr")
                    nc.scalar.activation(
                        out=zr, in_=ps, func=mybir.ActivationFunctionType.Relu,
                        bias=b3[:, oc], scale=1.0,
                    )
                    nc.sync.dma_start(out=out_d[b, oc], in_=zr)

    nc.compile()
    return nc


_PROG = None


def _host_prep(inputs):
    import ml_dtypes
    bf = ml_dtypes.bfloat16
    f = lambda a: np.asarray(a, dtype=np.float32)
    x = f(inputs["x"])
    # fold bn scales into weights (bn(conv(x,W),s,b) = conv(x, s*W) + b)
    w1 = f(inputs["w_conv1"]) * f(inputs["bn1_s"])[:, None]
    wq = f(inputs["wq"]) * f(inputs["bnq_s"])[:, None]
    wk = f(inputs["wk"]) * f(inputs["bnk_s"])[:, None]
    # fold bnatt scale through the (linear) attention-value path into v
    sv = f(inputs["bnatt_s"]) * f(inputs["bnv_s"])
    wv = f(inputs["wv"]) * sv[:, None]
    bv = f(inputs["bnatt_s"]) * f(inputs["bnv_b"])
    w3 = f(inputs["w_conv3"]) * f(inputs["bn3_s"])[:, None]

    posf = (f(inputs["pos_h"]) + f(inputs["pos_w"])).reshape(WIDTH, NKK)

    sel = np.zeros((PT, P, HEADS), np.float32)
    for pt in range(P // 64):
        pass
    for pt in range(PT):
        for c in range(P):
            sel[pt, c, pt * (P // D) + c // D] = 1.0
    # p2[pt, c, t, 32*j+g] = pos[c_global, 4t+j] if head(c_global)==g
    p2 = np.zeros((PT, P, NT, P), np.float32)
    for pt in range(PT):
        for c in range(P):
            g = pt * (P // D) + c // D
            for kk in range(NKK):
                t, j = kk // 4, kk % 4
                p2[pt, c, t, 32 * j + g] = posf[pt * P + c, kk]
    # sab[r, g] = 1 if r % 32 == g (sum over the 4 packed kk rows)
    sab = np.zeros((P, HEADS), np.float32)
    for r in range(P):
        sab[r, r % HEADS] = 1.0
    com = {
        "w1T": np.ascontiguousarray(w1.T.reshape(KC1, P, WIDTH)).astype(bf),
        "wqT": np.ascontiguousarray(wq.T.reshape(PT, P, WIDTH)).astype(bf),
        "wkT": np.ascontiguousarray(wk.T.reshape(PT, P, WIDTH)).astype(bf),
        "wvT": np.ascontiguousarray(wv.T.reshape(PT, P, WIDTH)).astype(bf),
        "w3T": np.ascontiguousarray(w3.T.reshape(PT, P, OUT)).astype(bf),
        "b1": f(inputs["bn1_b"]).reshape(PT, P, 1),
        "bq": f(inputs["bnq_b"]).reshape(PT, P, 1),
        "bk": f(inputs["bnk_b"]).reshape(PT, P, 1),
        "bv": bv.reshape(PT, P, 1),
        "batt": f(inputs["bnatt_b"]).reshape(PT, P, 1),
        "b3": f(inputs["bn3_b"]).reshape(OC, P, 1),
        "sel": sel.astype(bf),
        "p2": p2.astype(bf),
        "sab": sab.astype(bf),
        "eye32": np.eye(HEADS, dtype=np.float32).astype(bf),
        "ident": np.eye(P, dtype=np.float32).astype(bf),
    }
    xr = x.reshape(B, KC1, P, HW)
    in_maps = []
    for c in range(NC_):
        xs = np.ascontiguousarray(xr[c * BL:(c + 1) * BL])
        in_maps.append(dict(com, x16=xs.astype(bf)))
    return in_maps


def kernel(**inputs):
    global _PROG
    if _PROG is None:
        _PROG = build_program()
    in_maps = _host_prep(inputs)
    res = run_bass_kernel_spmd(_PROG, in_maps, core_ids=list(range(NC_)))
    outs = [res.results[c]["out"].reshape(BL, OUT, H, W) for c in range(NC_)]
    return np.concatenate(outs, axis=0)
